# revision 6
# baseline (speedup 1.0000x reference)
"""DEM particle-force stencil (125-shift 5x5x5 neighborhood) on 8 TRN2 NeuronCores.

Self-contained: hardcodes shapes/sharding. kernel(**inputs) takes full-size
numpy arrays, shards across 8 cores with halos baked host-side, runs a Bass
kernel per core (SPMD, no collectives), and reassembles the full output.
"""
import math
import numpy as np

# ---------------- problem constants ----------------
N = 192
CORES = 8
KN = 500000.0
MU = 0.5
EPS = 1e-4
_alpha = -math.log(0.7) / math.pi
_gamma = _alpha / math.sqrt(_alpha**2 + 1.0)
ETA = 2.0 * _gamma * math.sqrt(KN * 1.0)

# ---------------- layout constants ----------------
BI, BJ, BK = 3, 12, 8          # interior block per lane
IBLK, JBLK = 8, 16             # 8*16 = 128 lanes; covers 24 x 192 (i x j) per core
NIP = N // CORES               # 24 i-planes per core
ROUNDS = N // BK               # 24 k-rounds
FD = BI * BJ * BK              # 288 interior elems per lane per round
HI, HJ, HK = BI + 4, BJ + 4, BK + 4   # 7,16,12 haloed block
BLKE = HI * HJ * HK            # 1344
SI, SJ = HJ * HK, HK           # free-dim strides of haloed block: 192, 12
NAMES = ["x", "y", "z", "vx", "vy", "vz"]

_cache = {}


def _build_nc(two_d):
    import concourse.bass as bass
    import concourse.tile as tile
    import bass_rust
    from concourse import mybir

    F32 = mybir.dt.float32
    F32R = mybir.dt.float32r
    AF = mybir.ActivationFunctionType
    OP = mybir.AluOpType
    fourd2 = float(two_d * two_d)

    nc = bass.Bass(target_bir_lowering=False, debug=False)
    dram_in = {
        nm: nc.declare_dram_parameter(nm, [ROUNDS, 128, BLKE], F32, isOutput=False)
        for nm in NAMES
    }
    ident_d = nc.declare_dram_parameter("ident", [128, 128], F32, isOutput=False)
    out_d = nc.declare_dram_parameter("out", [ROUNDS, 9, 128, FD], F32, isOutput=True)

    def part_pair(ap):
        return list(ap.ap.to_list()[0])

    def wini(t, s0, s1, koff, B, di):
        """Shifted-window read (one interior i-plane di) for shifts (s0,s1),
        k-batch of B offsets starting at koff. 3 free dims: (batch, j, k)."""
        a = t[:].copy()
        a.ap = bass_rust.VecI64Pair(
            [part_pair(t[:]), [1, B], [SJ, BJ], [1, BK]]
        )
        a.offset = (2 - s0 + di) * SI + (2 - s1) * SJ + koff
        return a

    def basei(t, B, di):
        """Unshifted (center) read of i-plane di, broadcast across the batch."""
        a = t[:].copy()
        a.ap = bass_rust.VecI64Pair(
            [part_pair(t[:]), [0, B], [SJ, BJ], [1, BK]]
        )
        a.offset = (2 + di) * SI + 2 * SJ + 2
        return a

    def tvi(t, B, di):
        """[B, BJ, BK] view of temp tile at i-plane di (strides FD, BK, 1)."""
        a = t[:].copy()
        a.ap = bass_rust.VecI64Pair(
            [part_pair(t[:]), [FD, B], [BK, BJ], [1, BK]]
        )
        a.offset = di * BJ * BK
        return a

    # shift groups: (s0, s1) in [-2,2]^2; k-batches over s2.
    # batch m=0..B-1 corresponds to s2 = s2_hi - m, read k-offset = (2 - s2_hi) + m
    groups = []
    for s0 in range(-2, 3):
        for s1 in range(-2, 3):
            if s0 == 0 and s1 == 0:
                groups.append((0, 0, 2, 2))    # s2 in {2,1}
                groups.append((0, 0, -1, 2))   # s2 in {-1,-2}
            else:
                groups.append((s0, s1, 2, 5))  # s2 in {2..-2}

    # temp slot plan: tag -> bufs (all sized for B=5 * FD)
    TAGS = {
        "dx": 2, "dy": 2, "dz": 2, "dvx": 1, "dvy": 1, "dvz": 1,
        "q": 2, "sx": 1, "sy": 1, "sz": 1, "v": 1, "rr": 1, "cv": 1,
        "cs": 1, "bs": 1, "p1": 1, "p2": 1, "p3": 1,
    }

    with tile.TileContext(nc) as tc:
        import contextlib
        with contextlib.ExitStack() as ctx:
            ipool = ctx.enter_context(tc.tile_pool(name="in", bufs=1))
            dpool = ctx.enter_context(tc.tile_pool(name="tmp", bufs=1))
            opool = ctx.enter_context(tc.tile_pool(name="out", bufs=2))
            ppool = ctx.enter_context(tc.tile_pool(name="psum", bufs=1, space="PSUM"))
            cpool = ctx.enter_context(tc.tile_pool(name="const", bufs=1))

            ident_f = cpool.tile([128, 128], F32, name="ident_f")
            nc.sync.dma_start(ident_f[:], ident_d.ap())
            ident_s = cpool.tile([128, 128], F32R, name="ident_s")
            nc.vector.tensor_copy(ident_s[:], ident_f[:])

            def body(r):
                tin = {}
                for nm in NAMES:
                    t = ipool.tile([128, BLKE], F32, tag=nm, name=f"in_{nm}")
                    src = dram_in[nm].ap()[bass.ds(r, 1), :, :]
                    nc.sync.dma_start(t[:].rearrange("p (u f) -> p u f", u=1), src)
                    tin[nm] = t

                accs = [ppool.tile([128, FD], F32, tag=f"acc{i}", name=f"acc{i}")
                        for i in range(6)]
                nmm = [0] * 6  # matmul counter per accumulator
                total_sh = 124

                def accum(ai, src_ap):
                    i = nmm[ai]
                    nc.tensor.matmul(
                        accs[ai][:],
                        ident_s[:],
                        src_ap,
                        start=(i == 0),
                        stop=(i == total_sh - 1),
                    )
                    nmm[ai] += 1

                V, G, A = nc.vector, nc.gpsimd, nc.scalar

                def mkt(tag, B, name, dtype=F32):
                    # allocate a [128, B*FD] temp in slot `tag` (slot sized 5*FD)
                    return dpool.tile([128, B * FD], dtype, tag=tag,
                                      bufs=TAGS[tag], name=name,
                                      padded_shape=[128, 5 * FD])

                for (s0, s1, s2hi, B) in groups:
                    def tt(tag, name):
                        return mkt(tag, B, name)
                    dx = tt("dx", "dx")
                    dy = tt("dy", "dy")
                    dz = tt("dz", "dz")
                    for di in range(BI):
                        V.tensor_tensor(tvi(dx, B, di), basei(tin["x"], B, di), wini(tin["x"], s0, s1, 2 - s2hi, B, di), OP.subtract)
                    for di in range(BI):
                        V.tensor_tensor(tvi(dy, B, di), basei(tin["y"], B, di), wini(tin["y"], s0, s1, 2 - s2hi, B, di), OP.subtract)
                    for di in range(BI):
                        V.tensor_tensor(tvi(dz, B, di), basei(tin["z"], B, di), wini(tin["z"], s0, s1, 2 - s2hi, B, di), OP.subtract)
                    sx = tt("sx", "sx")
                    sy = tt("sy", "sy")
                    sz = tt("sz", "sz")
                    A.activation(sx[:], dx[:], AF.Square)
                    A.activation(sy[:], dy[:], AF.Square)
                    A.activation(sz[:], dz[:], AF.Square)
                    q = tt("q", "q")
                    G.tensor_tensor(sx[:], sx[:], sy[:], OP.add)
                    G.tensor_tensor(q[:], sx[:], sz[:], OP.add)
                    # r = rsqrt(q) via exp(-0.5 ln q), one Newton step
                    l = tt("sx", "l")
                    A.activation(l[:], q[:], AF.Ln)
                    r0 = tt("sy", "r0")
                    A.activation(r0[:], l[:], AF.Exp, scale=-0.5)
                    u = tt("sz", "u")
                    A.activation(u[:], r0[:], AF.Square)
                    v = tt("v", "v")
                    V.tensor_tensor(v[:], q[:], u[:], OP.mult)
                    V.tensor_scalar(v[:], v[:], -0.5, 1.5, OP.mult, OP.add)
                    rr = tt("rr", "rr")
                    V.tensor_tensor(rr[:], r0[:], v[:], OP.mult)
                    # collision scale cs = (q < (2d)^2) * (1 - 2d*r)
                    cv = tt("cv", "cv")
                    V.tensor_scalar(cv[:], rr[:], -float(two_d), 1.0, OP.mult, OP.add)
                    cs = tt("cs", "cs")
                    V.scalar_tensor_tensor(cs[:], q[:], fourd2, cv[:], OP.is_lt, OP.mult)
                    # velocity diffs and dot product
                    dvx = tt("dvx", "dvx")
                    dvy = tt("dvy", "dvy")
                    dvz = tt("dvz", "dvz")
                    for di in range(BI):
                        V.tensor_tensor(tvi(dvx, B, di), basei(tin["vx"], B, di), wini(tin["vx"], s0, s1, 2 - s2hi, B, di), OP.subtract)
                    for di in range(BI):
                        V.tensor_tensor(tvi(dvy, B, di), basei(tin["vy"], B, di), wini(tin["vy"], s0, s1, 2 - s2hi, B, di), OP.subtract)
                    for di in range(BI):
                        V.tensor_tensor(tvi(dvz, B, di), basei(tin["vz"], B, di), wini(tin["vz"], s0, s1, 2 - s2hi, B, di), OP.subtract)
                    p1 = tt("p1", "p1")
                    p2 = tt("p2", "p2")
                    p3 = tt("p3", "p3")
                    V.tensor_tensor(p1[:], dvx[:], dx[:], OP.mult)
                    V.tensor_tensor(p2[:], dvy[:], dy[:], OP.mult)
                    G.tensor_tensor(p3[:], dvz[:], dz[:], OP.mult)
                    G.tensor_tensor(p1[:], p1[:], p2[:], OP.add)
                    G.tensor_tensor(p1[:], p1[:], p3[:], OP.add)
                    r2 = tt("sx", "r2")
                    A.activation(r2[:], rr[:], AF.Square)
                    wd = tt("cv", "wd")
                    V.tensor_tensor(wd[:], p1[:], r2[:], OP.mult)
                    bs = tt("bs", "bs")
                    V.scalar_tensor_tensor(bs[:], q[:], fourd2, wd[:], OP.is_lt, OP.mult)
                    # force fields and PE accumulation
                    cx = mkt("dvx", B, "cx", F32R)
                    cy = mkt("dvy", B, "cy", F32R)
                    cz = mkt("dvz", B, "cz", F32R)
                    ex = mkt("p2", B, "ex", F32R)
                    ey = mkt("p3", B, "ey", F32R)
                    ez = mkt("sz", B, "ez", F32R)
                    V.tensor_tensor(cx[:], cs[:], dx[:], OP.mult)
                    V.tensor_tensor(cy[:], cs[:], dy[:], OP.mult)
                    V.tensor_tensor(cz[:], cs[:], dz[:], OP.mult)
                    G.tensor_tensor(ex[:], bs[:], dx[:], OP.mult)
                    G.tensor_tensor(ey[:], bs[:], dy[:], OP.mult)
                    G.tensor_tensor(ez[:], bs[:], dz[:], OP.mult)
                    for m in range(B):
                        sl = slice(m * FD, (m + 1) * FD)
                        accum(0, cx[:, sl])
                        accum(1, cy[:, sl])
                        accum(2, cz[:, sl])
                        accum(3, ex[:, sl])
                        accum(4, ey[:, sl])
                        accum(5, ez[:, sl])

                # ---- scale accumulators to output force values ----
                outs = []
                for o in range(6):
                    ot = opool.tile([128, FD], F32, tag=f"o{o}", name=f"o{o}")
                    A.activation(ot[:], accs[o][:], AF.Copy, scale=(KN if o < 3 else ETA))
                    outs.append(ot)

                # ---- friction from last shift (2,2,2) ----
                # temps reuse the main group slots (all groups are done here)
                dxl = mkt("dx", 1, "dxl")
                dyl = mkt("dy", 1, "dyl")
                dzl = mkt("dz", 1, "dzl")
                for di in range(BI):
                    V.tensor_tensor(tvi(dxl, 1, di), basei(tin["x"], 1, di), wini(tin["x"], 2, 2, 0, 1, di), OP.subtract)
                for di in range(BI):
                    V.tensor_tensor(tvi(dyl, 1, di), basei(tin["y"], 1, di), wini(tin["y"], 2, 2, 0, 1, di), OP.subtract)
                for di in range(BI):
                    V.tensor_tensor(tvi(dzl, 1, di), basei(tin["z"], 1, di), wini(tin["z"], 2, 2, 0, 1, di), OP.subtract)
                t1 = mkt("sx", 1, "t1")
                t2 = mkt("sy", 1, "t2")
                ql = mkt("q", 1, "ql")
                A.activation(t1[:], dxl[:], AF.Square)
                A.activation(t2[:], dyl[:], AF.Square)
                A.activation(ql[:], dzl[:], AF.Square)
                G.tensor_tensor(t1[:], t1[:], t2[:], OP.add)
                G.tensor_tensor(ql[:], t1[:], ql[:], OP.add)
                mneg = mkt("cs", 1, "mneg")  # -(q < (2d)^2)
                V.tensor_scalar(mneg[:], ql[:], fourd2, -1.0, OP.is_lt, OP.mult)
                dvxl = mkt("dvx", 1, "dvxl")
                dvyl = mkt("dvy", 1, "dvyl")
                dvzl = mkt("dvz", 1, "dvzl")
                for di in range(BI):
                    V.tensor_tensor(tvi(dvxl, 1, di), basei(tin["vx"], 1, di), wini(tin["vx"], 2, 2, 0, 1, di), OP.subtract)
                for di in range(BI):
                    V.tensor_tensor(tvi(dvyl, 1, di), basei(tin["vy"], 1, di), wini(tin["vy"], 2, 2, 0, 1, di), OP.subtract)
                for di in range(BI):
                    V.tensor_tensor(tvi(dvzl, 1, di), basei(tin["vz"], 1, di), wini(tin["vz"], 2, 2, 0, 1, di), OP.subtract)

                def safe_recip_abs(dv, rtag, idx):
                    # 1 / max(EPS, |dv|), exp/ln + 2 Newton steps
                    aa = mkt("sz", 1, f"aa{idx}")
                    A.activation(aa[:], dv[:], AF.Abs)
                    V.tensor_scalar(aa[:], aa[:], EPS, None, OP.max)
                    ll = mkt("v", 1, f"ll{idx}")
                    A.activation(ll[:], aa[:], AF.Ln)
                    rr0 = mkt(rtag, 1, f"rcp{idx}")
                    A.activation(rr0[:], ll[:], AF.Exp, scale=-1.0)
                    for it in range(2):
                        tn = mkt("cv", 1, f"tn{idx}_{it}")
                        V.tensor_tensor(tn[:], aa[:], rr0[:], OP.mult)
                        V.tensor_scalar(tn[:], tn[:], -1.0, 2.0, OP.mult, OP.add)
                        V.tensor_tensor(rr0[:], rr0[:], tn[:], OP.mult)
                    return rr0

                rx = safe_recip_abs(dvxl, "p1", 0)
                ry = safe_recip_abs(dvyl, "p2", 1)
                rz = safe_recip_abs(dvzl, "p3", 2)
                # numerator factors: dvx/|dvx|_safe, dvy/|dvy|_safe, dvy/|dvz|_safe
                fax = mkt("rr", 1, "fax")
                fay = mkt("bs", 1, "fay")
                faz = mkt("dx", 1, "faz")
                V.tensor_tensor(fax[:], dvxl[:], rx[:], OP.mult)
                V.tensor_tensor(fay[:], dvyl[:], ry[:], OP.mult)
                V.tensor_tensor(faz[:], dvyl[:], rz[:], OP.mult)

                # |f_c| of scaled collision forces
                afx = mkt("dy", 1, "afx")
                afy = mkt("dz", 1, "afy")
                afz = mkt("sx", 1, "afz")
                A.activation(afx[:], outs[0][:], AF.Abs)
                A.activation(afy[:], outs[1][:], AF.Abs)
                A.activation(afz[:], outs[2][:], AF.Abs)

                fr_defs = [
                    (afy, afz, outs[3], fax),  # frx: |fyc|+|fzc| - fxd, dir x
                    (afx, afz, outs[4], fay),  # fry: |fxc|+|fzc| - fyd, dir y
                    (afx, afy, outs[5], faz),  # frz: |fxc|+|fyc| - fzd, dir (vy/|vz|)
                ]
                for o, (a1, a2, dmp, fac) in enumerate(fr_defs):
                    u1 = mkt("sy", 1, f"u1_{o}")
                    G.tensor_tensor(u1[:], a1[:], a2[:], OP.add)
                    G.tensor_tensor(u1[:], u1[:], dmp[:], OP.subtract)
                    u2 = mkt("q", 1, f"u2_{o}")
                    A.activation(u2[:], u1[:], AF.Abs, scale=MU)
                    V.tensor_tensor(u2[:], u2[:], fac[:], OP.mult)
                    ot = opool.tile([128, FD], F32, tag=f"o{6+o}", name=f"o{6+o}")
                    V.tensor_tensor(ot[:], u2[:], mneg[:], OP.mult)
                    outs.append(ot)

                for o in range(9):
                    dst = out_d.ap()[bass.ds(r, 1), o, :, :]
                    nc.sync.dma_start(dst, outs[o][:].rearrange("p (u f) -> p u f", u=1))

            with tc.For_i(0, ROUNDS, 1) as r:
                body(r)

    return nc


# ---------------- host-side sharding ----------------

def _split_excess_waits(nc, max_waits=1):
    """This walrus build allows only max_waits semaphore waits per instruction;
    hoist the excess onto NoOps inserted just before the offender."""
    from concourse import mybir
    cnt = 0
    for blk in nc.m.functions[0].blocks:
        new = []
        changed = False
        for ins in blk.instructions:
            si = ins.sync_info
            if si is not None and si.on_wait and len(si.on_wait) > max_waits:
                waits = list(si.on_wait)
                keep = waits[-max_waits:]
                extra = waits[:-max_waits]
                for i in range(0, len(extra), max_waits):
                    nop = mybir.InstNoOp(name=f"wait_split_{cnt}", ins=[], outs=[])
                    cnt += 1
                    nop.engine = ins.engine
                    nop.sync_info = type(si)(on_wait=extra[i:i + max_waits], on_update=[])
                    new.append(nop)
                ins.sync_info = type(si)(on_wait=keep, on_update=si.on_update)
                changed = True
            new.append(ins)
        if changed:
            blk.instructions = new
    return nc


def _prep_in_maps(inputs):
    from numpy.lib.stride_tricks import sliding_window_view

    arrs = {
        "x": inputs["x_grid"], "y": inputs["y_grid"], "z": inputs["z_grid"],
        "vx": inputs["vx_grid"], "vy": inputs["vy_grid"], "vz": inputs["vz_grid"],
    }
    lanes = np.arange(128)
    j_starts = (lanes % JBLK) * BJ                 # 0..180
    k_starts = np.arange(ROUNDS) * BK              # 0..184
    ident = np.eye(128, dtype=np.float32)

    in_maps = [dict() for _ in range(CORES)]
    for nm, a in arrs.items():
        ap = np.pad(np.asarray(a, dtype=np.float32), 2, mode="wrap")  # [196]^3
        W = sliding_window_view(ap, (HI, HJ, HK))  # [190,181,185,7,16,12] view
        for c in range(CORES):
            i_starts = c * NIP + (lanes // JBLK) * BI
            blk = W[i_starts[None, :], j_starts[None, :], k_starts[:, None]]
            in_maps[c][nm] = np.ascontiguousarray(
                blk.reshape(ROUNDS, 128, BLKE), dtype=np.float32)
    for c in range(CORES):
        in_maps[c]["ident"] = ident
    return in_maps


def _gather(results):
    out = np.empty((9, N, N, N), dtype=np.float32)
    for c in range(CORES):
        r = results[c]["out"].reshape(ROUNDS, 9, IBLK, JBLK, BI, BJ, BK)
        # [r, o, ib, jb, di, dj, dk] -> [o, ib, di, jb, dj, r, dk]
        blk = r.transpose(1, 2, 4, 3, 5, 0, 6).reshape(9, NIP, N, N)
        out[:, c * NIP:(c + 1) * NIP] = blk
    return out


def _run(inputs, trace=False):
    from concourse.bass_utils import run_bass_kernel_spmd

    d = float(np.asarray(inputs["d"]))
    two_d = 2.0 * d
    key = round(two_d, 9)
    if key not in _cache:
        _cache[key] = _split_excess_waits(_build_nc(two_d))
    nc = _cache[key]
    in_maps = _prep_in_maps(inputs)
    res = run_bass_kernel_spmd(nc, in_maps, core_ids=list(range(CORES)), trace=trace)
    return _gather(res.results), res


def kernel(**inputs):
    out, _ = _run(inputs, trace=False)
    return out


# revision 15
# speedup vs baseline: 2.2136x; 2.2136x over previous
"""DEM particle-force stencil (125-shift 5x5x5 neighborhood) on 8 TRN2 NeuronCores.

Self-contained: hardcodes shapes/sharding. kernel(**inputs) takes full-size
numpy arrays, shards across 8 cores with halos baked host-side, runs a Bass
kernel per core (SPMD, no collectives), and reassembles the full output.
"""
import math
import numpy as np

# ---------------- problem constants ----------------
N = 192
CORES = 8
KN = 500000.0
MU = 0.5
EPS = 1e-4
_alpha = -math.log(0.7) / math.pi
_gamma = _alpha / math.sqrt(_alpha**2 + 1.0)
ETA = 2.0 * _gamma * math.sqrt(KN * 1.0)

# ---------------- layout constants ----------------
BI, BJ, BK = 3, 12, 8          # interior block per lane
IBLK, JBLK = 8, 16             # 8*16 = 128 lanes; covers 24 x 192 (i x j) per core
NIP = N // CORES               # 24 i-planes per core
ROUNDS = N // BK               # 24 k-rounds
FD = BI * BJ * BK              # 288 interior elems per lane per round
HI, HJ, HK = BI + 4, BJ + 4, BK + 4   # 7,16,12 haloed block
BLKE = HI * HJ * HK            # 1344
SI, SJ = HJ * HK, HK           # free-dim strides of haloed block: 192, 12
NAMES = ["x", "y", "z", "vx", "vy", "vz"]

_cache = {}


def _build_nc(two_d):
    import concourse.bass as bass
    import concourse.tile as tile
    import bass_rust
    from concourse import mybir, bass_isa

    F32 = mybir.dt.float32
    F32R = mybir.dt.float32r
    AF = mybir.ActivationFunctionType
    OP = mybir.AluOpType
    fourd2 = float(two_d * two_d)

    nc = bass.Bass(target_bir_lowering=False, debug=False)
    dram_in = {
        nm: nc.declare_dram_parameter(nm, [ROUNDS, 128, BLKE], F32, isOutput=False)
        for nm in NAMES
    }
    ident_d = nc.declare_dram_parameter("ident", [128, 128], F32, isOutput=False)
    out_d = nc.declare_dram_parameter("out", [ROUNDS, 9, 128, FD], F32, isOutput=True)

    def part_pair(ap):
        return list(ap.ap.to_list()[0])

    def wini(t, s0, s1, koff, B, di):
        """Shifted-window read (one interior i-plane di) for shifts (s0,s1),
        k-batch of B offsets starting at koff. 3 free dims: (batch, j, k)."""
        a = t[:].copy()
        a.ap = bass_rust.VecI64Pair(
            [part_pair(t[:]), [1, B], [SJ, BJ], [1, BK]]
        )
        a.offset = (2 - s0 + di) * SI + (2 - s1) * SJ + koff
        return a

    def basei(t, B, di):
        """Unshifted (center) read of i-plane di, broadcast across the batch."""
        a = t[:].copy()
        a.ap = bass_rust.VecI64Pair(
            [part_pair(t[:]), [0, B], [SJ, BJ], [1, BK]]
        )
        a.offset = (2 + di) * SI + 2 * SJ + 2
        return a

    def tvi(t, B, di):
        """[B, BJ, BK] view of temp tile at i-plane di (strides FD, BK, 1)."""
        a = t[:].copy()
        a.ap = bass_rust.VecI64Pair(
            [part_pair(t[:]), [FD, B], [BK, BJ], [1, BK]]
        )
        a.offset = di * BJ * BK
        return a

    # shift groups: (s0, s1) in [-2,2]^2; k-batches over s2.
    # batch m=0..B-1 corresponds to s2 = s2_hi - m, read k-offset = (2 - s2_hi) + m
    groups = []
    for s0 in range(-2, 3):
        for s1 in range(-2, 3):
            if s0 == 0 and s1 == 0:
                groups.append((0, 0, 2, 2))    # s2 in {2,1}
                groups.append((0, 0, -1, 2))   # s2 in {-1,-2}
            else:
                groups.append((s0, s1, 2, 5))  # s2 in {2..-2}

    # temp slot plan: tag -> bufs (all sized for B=5 * FD)
    TAGS = {
        "dx": 2, "dy": 2, "dz": 2, "dvx": 1, "dvy": 1, "dvz": 1,
        "q": 2, "sx": 1, "sy": 1, "sz": 1, "v": 1, "rr": 1, "cv": 1,
        "cs": 1, "bs": 1, "p1": 1, "p2": 1, "p3": 1,
    }

    with tile.TileContext(nc) as tc:
        import contextlib
        with contextlib.ExitStack() as ctx:
            ipool = ctx.enter_context(tc.tile_pool(name="in", bufs=1))
            dpool = ctx.enter_context(tc.tile_pool(name="tmp", bufs=1))
            opool = ctx.enter_context(tc.tile_pool(name="out", bufs=2))
            ppool = ctx.enter_context(tc.tile_pool(name="psum", bufs=1, space="PSUM"))
            cpool = ctx.enter_context(tc.tile_pool(name="const", bufs=1))
            spool = ctx.enter_context(tc.tile_pool(name="small", bufs=2))

            ident_f = cpool.tile([128, 128], F32, name="ident_f")
            nc.sync.dma_start(ident_f[:], ident_d.ap())
            ident_s = cpool.tile([128, 128], F32R, name="ident_s")
            nc.vector.tensor_copy(ident_s[:], ident_f[:])
            zero_f = cpool.tile([128, FD], F32, name="zero_f")
            nc.vector.memset(zero_f[:], 0.0)
            zero_r = cpool.tile([128, FD], F32R, name="zero_r")
            nc.vector.tensor_copy(zero_r[:], zero_f[:])
            ones_f = cpool.tile([128, 1], F32, name="ones_f")
            nc.vector.memset(ones_f[:], 1.0)
            OrderedSet = bass.OrderedSet
            skip_regs = nc.alloc_registers(
                "skipregs",
                OrderedSet([mybir.EngineType.DVE, mybir.EngineType.Pool,
                            mybir.EngineType.Activation, mybir.EngineType.PE]),
            )
            NEG_FOURD2_BITS = int(np.float32(-fourd2).view(np.int32))

            def body(r):
                tin = {}
                for nm in NAMES:
                    t = ipool.tile([128, BLKE], F32, tag=nm, name=f"in_{nm}")
                    src = dram_in[nm].ap()[bass.ds(r, 1), :, :]
                    nc.sync.dma_start(t[:].rearrange("p (u f) -> p u f", u=1), src)
                    tin[nm] = t

                accs = [ppool.tile([128, FD], F32, tag=f"acc{i}", name=f"acc{i}")
                        for i in range(6)]
                for ai in range(6):  # open accumulation groups (clear PSUM)
                    nc.tensor.matmul(accs[ai][:], ident_s[:], zero_r[:],
                                     start=True, stop=False, skip_group_check=True)

                def accum(ai, src_ap):
                    nc.tensor.matmul(accs[ai][:], ident_s[:], src_ap,
                                     start=False, stop=False, skip_group_check=True)

                V, G, A = nc.vector, nc.gpsimd, nc.scalar

                def mkt(tag, B, name, dtype=F32):
                    # allocate a [128, B*FD] temp in slot `tag` (slot sized 5*FD)
                    return dpool.tile([128, B * FD], dtype, tag=tag,
                                      bufs=TAGS[tag], name=name,
                                      padded_shape=[128, 5 * FD])

                spool_i = [0]

                def small(name):
                    spool_i[0] += 1
                    return spool.tile([128, 1], F32, tag=f"s{spool_i[0] % 4}",
                                      name=f"{name}{spool_i[0]}")

                for (s0, s1, s2hi, B) in groups:
                    def tt(tag, name):
                        return mkt(tag, B, name)
                    # ---------- always: position diffs, q, hit test ----------
                    dx = tt("dx", "dx")
                    dy = tt("dy", "dy")
                    dz = tt("dz", "dz")
                    for di in range(BI):
                        V.tensor_tensor(tvi(dx, B, di), basei(tin["x"], B, di), wini(tin["x"], s0, s1, 2 - s2hi, B, di), OP.subtract)
                    for di in range(BI):
                        V.tensor_tensor(tvi(dy, B, di), basei(tin["y"], B, di), wini(tin["y"], s0, s1, 2 - s2hi, B, di), OP.subtract)
                    for di in range(BI):
                        V.tensor_tensor(tvi(dz, B, di), basei(tin["z"], B, di), wini(tin["z"], s0, s1, 2 - s2hi, B, di), OP.subtract)
                    sx = tt("sx", "sx")
                    sy = tt("sy", "sy")
                    sz = tt("sz", "sz")
                    A.activation(sx[:], dx[:], AF.Square)
                    A.activation(sy[:], dy[:], AF.Square)
                    A.activation(sz[:], dz[:], AF.Square)
                    q = tt("q", "q")
                    G.tensor_tensor(sx[:], sx[:], sy[:], OP.add)
                    G.tensor_tensor(q[:], sx[:], sz[:], OP.add)
                    junk = mkt("cs", B, "junk")
                    msum = small("msum")
                    V.tensor_scalar(junk[:], q[:], fourd2, 0.0, OP.is_lt, OP.add,
                                    accum_out=msum[:])
                    pp = ppool.tile([1, 1], F32, tag="hit", name="hit")
                    nc.tensor.matmul(pp[:1, :1], ones_f[:, :1], msum[:, :1],
                                     start=True, stop=True, skip_group_check=True)
                    hits = small("hits")
                    A.activation(hits[:1, :1], pp[:1, :1], AF.Copy, scale=1.0)
                    nc.regs_load(skip_regs, hits[:1, :1].bitcast(mybir.dt.int32))

                    with tc.If(nc.snap(skip_regs) > 0):
                        # ---------- hot path: forces for this shift group ----------
                        l = tt("sx", "l")
                        A.activation(l[:], q[:], AF.Ln)
                        r0 = tt("sy", "r0")
                        A.activation(r0[:], l[:], AF.Exp, scale=-0.5)
                        u = tt("sz", "u")
                        A.activation(u[:], r0[:], AF.Square)
                        v = tt("v", "v")
                        V.tensor_tensor(v[:], q[:], u[:], OP.mult)
                        A.activation(v[:], v[:], AF.Copy, bias=1.5, scale=-0.5)
                        rr = tt("rr", "rr")
                        V.tensor_tensor(rr[:], r0[:], v[:], OP.mult)
                        # collision scale cs = (q < (2d)^2) * (1 - 2d*r)
                        cv = tt("cv", "cv")
                        A.activation(cv[:], rr[:], AF.Copy, bias=1.0, scale=-float(two_d))
                        cs = tt("cs", "cs")
                        V.scalar_tensor_tensor(cs[:], q[:], fourd2, cv[:], OP.is_lt, OP.mult)
                        # velocity diffs and dot product
                        dvx = tt("dvx", "dvx")
                        dvy = tt("dvy", "dvy")
                        dvz = tt("dvz", "dvz")
                        for di in range(BI):
                            V.tensor_tensor(tvi(dvx, B, di), basei(tin["vx"], B, di), wini(tin["vx"], s0, s1, 2 - s2hi, B, di), OP.subtract)
                        for di in range(BI):
                            V.tensor_tensor(tvi(dvy, B, di), basei(tin["vy"], B, di), wini(tin["vy"], s0, s1, 2 - s2hi, B, di), OP.subtract)
                        for di in range(BI):
                            V.tensor_tensor(tvi(dvz, B, di), basei(tin["vz"], B, di), wini(tin["vz"], s0, s1, 2 - s2hi, B, di), OP.subtract)
                        p1 = tt("p1", "p1")
                        p2 = tt("p2", "p2")
                        p3 = tt("p3", "p3")
                        V.tensor_tensor(p1[:], dvx[:], dx[:], OP.mult)
                        V.tensor_tensor(p2[:], dvy[:], dy[:], OP.mult)
                        G.tensor_tensor(p3[:], dvz[:], dz[:], OP.mult)
                        G.tensor_tensor(p1[:], p1[:], p2[:], OP.add)
                        G.tensor_tensor(p1[:], p1[:], p3[:], OP.add)
                        r2 = tt("sx", "r2")
                        A.activation(r2[:], rr[:], AF.Square)
                        wd = tt("cv", "wd")
                        V.tensor_tensor(wd[:], p1[:], r2[:], OP.mult)
                        bs = tt("bs", "bs")
                        V.scalar_tensor_tensor(bs[:], q[:], fourd2, wd[:], OP.is_lt, OP.mult)
                        # force fields and PE accumulation
                        cx = mkt("dvx", B, "cx", F32R)
                        cy = mkt("dvy", B, "cy", F32R)
                        cz = mkt("dvz", B, "cz", F32R)
                        ex = mkt("p2", B, "ex", F32R)
                        ey = mkt("p3", B, "ey", F32R)
                        ez = mkt("sz", B, "ez", F32R)
                        V.tensor_tensor(cx[:], cs[:], dx[:], OP.mult)
                        V.tensor_tensor(cy[:], cs[:], dy[:], OP.mult)
                        V.tensor_tensor(cz[:], cs[:], dz[:], OP.mult)
                        G.tensor_tensor(ex[:], bs[:], dx[:], OP.mult)
                        G.tensor_tensor(ey[:], bs[:], dy[:], OP.mult)
                        G.tensor_tensor(ez[:], bs[:], dz[:], OP.mult)
                        for m in range(B):
                            sl = slice(m * FD, (m + 1) * FD)
                            accum(0, cx[:, sl])
                            accum(1, cy[:, sl])
                            accum(2, cz[:, sl])
                            accum(3, ex[:, sl])
                            accum(4, ey[:, sl])
                            accum(5, ez[:, sl])

                for ai in range(6):  # close accumulation groups
                    nc.tensor.matmul(accs[ai][:], ident_s[:], zero_r[:],
                                     start=False, stop=True, skip_group_check=True)

                # ---- scale accumulators to output force values ----
                outs = []
                for o in range(6):
                    ot = opool.tile([128, FD], F32, tag=f"o{o}", name=f"o{o}")
                    A.activation(ot[:], accs[o][:], AF.Copy, scale=(KN if o < 3 else ETA))
                    outs.append(ot)

                # ---- friction from last shift (2,2,2) ----
                # temps reuse the main group slots (all groups are done here)
                dxl = mkt("dx", 1, "dxl")
                dyl = mkt("dy", 1, "dyl")
                dzl = mkt("dz", 1, "dzl")
                for di in range(BI):
                    V.tensor_tensor(tvi(dxl, 1, di), basei(tin["x"], 1, di), wini(tin["x"], 2, 2, 0, 1, di), OP.subtract)
                for di in range(BI):
                    V.tensor_tensor(tvi(dyl, 1, di), basei(tin["y"], 1, di), wini(tin["y"], 2, 2, 0, 1, di), OP.subtract)
                for di in range(BI):
                    V.tensor_tensor(tvi(dzl, 1, di), basei(tin["z"], 1, di), wini(tin["z"], 2, 2, 0, 1, di), OP.subtract)
                t1 = mkt("sx", 1, "t1")
                t2 = mkt("sy", 1, "t2")
                ql = mkt("q", 1, "ql")
                A.activation(t1[:], dxl[:], AF.Square)
                A.activation(t2[:], dyl[:], AF.Square)
                A.activation(ql[:], dzl[:], AF.Square)
                G.tensor_tensor(t1[:], t1[:], t2[:], OP.add)
                G.tensor_tensor(ql[:], t1[:], ql[:], OP.add)
                mneg = mkt("cs", 1, "mneg")  # -(q < (2d)^2)
                V.tensor_scalar(mneg[:], ql[:], fourd2, -1.0, OP.is_lt, OP.mult)
                dvxl = mkt("dvx", 1, "dvxl")
                dvyl = mkt("dvy", 1, "dvyl")
                dvzl = mkt("dvz", 1, "dvzl")
                for di in range(BI):
                    V.tensor_tensor(tvi(dvxl, 1, di), basei(tin["vx"], 1, di), wini(tin["vx"], 2, 2, 0, 1, di), OP.subtract)
                for di in range(BI):
                    V.tensor_tensor(tvi(dvyl, 1, di), basei(tin["vy"], 1, di), wini(tin["vy"], 2, 2, 0, 1, di), OP.subtract)
                for di in range(BI):
                    V.tensor_tensor(tvi(dvzl, 1, di), basei(tin["vz"], 1, di), wini(tin["vz"], 2, 2, 0, 1, di), OP.subtract)

                def safe_recip_abs(dv, rtag, idx):
                    # 1 / max(EPS, |dv|), exp/ln + 2 Newton steps
                    aa = mkt("sz", 1, f"aa{idx}")
                    A.activation(aa[:], dv[:], AF.Abs)
                    V.tensor_scalar(aa[:], aa[:], EPS, None, OP.max)
                    ll = mkt("v", 1, f"ll{idx}")
                    A.activation(ll[:], aa[:], AF.Ln)
                    rr0 = mkt(rtag, 1, f"rcp{idx}")
                    A.activation(rr0[:], ll[:], AF.Exp, scale=-1.0)
                    for it in range(2):
                        tn = mkt("cv", 1, f"tn{idx}_{it}")
                        V.tensor_tensor(tn[:], aa[:], rr0[:], OP.mult)
                        V.tensor_scalar(tn[:], tn[:], -1.0, 2.0, OP.mult, OP.add)
                        V.tensor_tensor(rr0[:], rr0[:], tn[:], OP.mult)
                    return rr0

                rx = safe_recip_abs(dvxl, "p1", 0)
                ry = safe_recip_abs(dvyl, "p2", 1)
                rz = safe_recip_abs(dvzl, "p3", 2)
                # numerator factors: dvx/|dvx|_safe, dvy/|dvy|_safe, dvy/|dvz|_safe
                fax = mkt("rr", 1, "fax")
                fay = mkt("bs", 1, "fay")
                faz = mkt("dx", 1, "faz")
                V.tensor_tensor(fax[:], dvxl[:], rx[:], OP.mult)
                V.tensor_tensor(fay[:], dvyl[:], ry[:], OP.mult)
                V.tensor_tensor(faz[:], dvyl[:], rz[:], OP.mult)

                # |f_c| of scaled collision forces
                afx = mkt("dy", 1, "afx")
                afy = mkt("dz", 1, "afy")
                afz = mkt("sx", 1, "afz")
                A.activation(afx[:], outs[0][:], AF.Abs)
                A.activation(afy[:], outs[1][:], AF.Abs)
                A.activation(afz[:], outs[2][:], AF.Abs)

                fr_defs = [
                    (afy, afz, outs[3], fax),  # frx: |fyc|+|fzc| - fxd, dir x
                    (afx, afz, outs[4], fay),  # fry: |fxc|+|fzc| - fyd, dir y
                    (afx, afy, outs[5], faz),  # frz: |fxc|+|fyc| - fzd, dir (vy/|vz|)
                ]
                for o, (a1, a2, dmp, fac) in enumerate(fr_defs):
                    u1 = mkt("sy", 1, f"u1_{o}")
                    G.tensor_tensor(u1[:], a1[:], a2[:], OP.add)
                    G.tensor_tensor(u1[:], u1[:], dmp[:], OP.subtract)
                    u2 = mkt("q", 1, f"u2_{o}")
                    A.activation(u2[:], u1[:], AF.Abs, scale=MU)
                    V.tensor_tensor(u2[:], u2[:], fac[:], OP.mult)
                    ot = opool.tile([128, FD], F32, tag=f"o{6+o}", name=f"o{6+o}")
                    V.tensor_tensor(ot[:], u2[:], mneg[:], OP.mult)
                    outs.append(ot)

                for o in range(9):
                    dst = out_d.ap()[bass.ds(r, 1), o, :, :]
                    nc.sync.dma_start(dst, outs[o][:].rearrange("p (u f) -> p u f", u=1))

            with tc.For_i(0, ROUNDS, 1) as r:
                body(r)

    return nc


# ---------------- host-side sharding ----------------

def _split_excess_waits(nc, max_waits=1):
    """This walrus build allows only max_waits semaphore waits per instruction;
    hoist the excess onto NoOps inserted just before the offender."""
    from concourse import mybir
    cnt = 0
    for blk in nc.m.functions[0].blocks:
        new = []
        changed = False
        for ins in blk.instructions:
            si = ins.sync_info
            if si is not None and si.on_wait and len(si.on_wait) > max_waits:
                waits = list(si.on_wait)
                keep = waits[-max_waits:]
                extra = waits[:-max_waits]
                for i in range(0, len(extra), max_waits):
                    nop = mybir.InstNoOp(name=f"wait_split_{cnt}", ins=[], outs=[])
                    cnt += 1
                    nop.engine = ins.engine
                    nop.sync_info = type(si)(on_wait=extra[i:i + max_waits], on_update=[])
                    new.append(nop)
                ins.sync_info = type(si)(on_wait=keep, on_update=si.on_update)
                changed = True
            new.append(ins)
        if changed:
            blk.instructions = new
    return nc


def _prep_in_maps(inputs):
    from numpy.lib.stride_tricks import sliding_window_view

    arrs = {
        "x": inputs["x_grid"], "y": inputs["y_grid"], "z": inputs["z_grid"],
        "vx": inputs["vx_grid"], "vy": inputs["vy_grid"], "vz": inputs["vz_grid"],
    }
    lanes = np.arange(128)
    j_starts = (lanes % JBLK) * BJ                 # 0..180
    k_starts = np.arange(ROUNDS) * BK              # 0..184
    ident = np.eye(128, dtype=np.float32)

    in_maps = [dict() for _ in range(CORES)]
    for nm, a in arrs.items():
        ap = np.pad(np.asarray(a, dtype=np.float32), 2, mode="wrap")  # [196]^3
        W = sliding_window_view(ap, (HI, HJ, HK))  # [190,181,185,7,16,12] view
        for c in range(CORES):
            i_starts = c * NIP + (lanes // JBLK) * BI
            blk = W[i_starts[None, :], j_starts[None, :], k_starts[:, None]]
            in_maps[c][nm] = np.ascontiguousarray(
                blk.reshape(ROUNDS, 128, BLKE), dtype=np.float32)
    for c in range(CORES):
        in_maps[c]["ident"] = ident
    return in_maps


def _gather(results):
    out = np.empty((9, N, N, N), dtype=np.float32)
    for c in range(CORES):
        r = results[c]["out"].reshape(ROUNDS, 9, IBLK, JBLK, BI, BJ, BK)
        # [r, o, ib, jb, di, dj, dk] -> [o, ib, di, jb, dj, r, dk]
        blk = r.transpose(1, 2, 4, 3, 5, 0, 6).reshape(9, NIP, N, N)
        out[:, c * NIP:(c + 1) * NIP] = blk
    return out


def _run(inputs, trace=False):
    from concourse.bass_utils import run_bass_kernel_spmd

    d = float(np.asarray(inputs["d"]))
    two_d = 2.0 * d
    key = round(two_d, 9)
    if key not in _cache:
        _cache[key] = _split_excess_waits(_build_nc(two_d))
    nc = _cache[key]
    in_maps = _prep_in_maps(inputs)
    res = run_bass_kernel_spmd(nc, in_maps, core_ids=list(range(CORES)), trace=trace)
    return _gather(res.results), res


def kernel(**inputs):
    out, _ = _run(inputs, trace=False)
    return out


# revision 16
# speedup vs baseline: 2.7424x; 1.2389x over previous
"""DEM particle-force stencil (125-shift 5x5x5 neighborhood) on 8 TRN2 NeuronCores.

Self-contained: hardcodes shapes/sharding. kernel(**inputs) takes full-size
numpy arrays, shards across 8 cores with halos baked host-side, runs a Bass
kernel per core (SPMD, no collectives), and reassembles the full output.
"""
import math
import numpy as np

# ---------------- problem constants ----------------
N = 192
CORES = 8
KN = 500000.0
MU = 0.5
EPS = 1e-4
_alpha = -math.log(0.7) / math.pi
_gamma = _alpha / math.sqrt(_alpha**2 + 1.0)
ETA = 2.0 * _gamma * math.sqrt(KN * 1.0)

# ---------------- layout constants ----------------
BI, BJ, BK = 3, 12, 8          # interior block per lane
IBLK, JBLK = 8, 16             # 8*16 = 128 lanes; covers 24 x 192 (i x j) per core
NIP = N // CORES               # 24 i-planes per core
ROUNDS = N // BK               # 24 k-rounds
FD = BI * BJ * BK              # 288 interior elems per lane per round
HI, HJ, HK = BI + 4, BJ + 4, BK + 4   # 7,16,12 haloed block
BLKE = HI * HJ * HK            # 1344
SI, SJ = HJ * HK, HK           # free-dim strides of haloed block: 192, 12
NAMES = ["x", "y", "z", "vx", "vy", "vz"]

_cache = {}


def _build_nc(two_d):
    import concourse.bass as bass
    import concourse.tile as tile
    import bass_rust
    from concourse import mybir, bass_isa

    F32 = mybir.dt.float32
    F32R = mybir.dt.float32r
    AF = mybir.ActivationFunctionType
    OP = mybir.AluOpType
    fourd2 = float(two_d * two_d)

    nc = bass.Bass(target_bir_lowering=False, debug=False)
    dram_in = {
        nm: nc.declare_dram_parameter(nm, [ROUNDS, 128, BLKE], F32, isOutput=False)
        for nm in NAMES
    }
    ident_d = nc.declare_dram_parameter("ident", [128, 128], F32, isOutput=False)
    out_d = nc.declare_dram_parameter("out", [ROUNDS, 9, 128, FD], F32, isOutput=True)

    def part_pair(ap):
        return list(ap.ap.to_list()[0])

    def wini(t, s0, s1, koff, B, di):
        """Shifted-window read (one interior i-plane di) for shifts (s0,s1),
        k-batch of B offsets starting at koff. 3 free dims: (batch, j, k)."""
        a = t[:].copy()
        a.ap = bass_rust.VecI64Pair(
            [part_pair(t[:]), [1, B], [SJ, BJ], [1, BK]]
        )
        a.offset = (2 - s0 + di) * SI + (2 - s1) * SJ + koff
        return a

    def basei(t, B, di):
        """Unshifted (center) read of i-plane di, broadcast across the batch."""
        a = t[:].copy()
        a.ap = bass_rust.VecI64Pair(
            [part_pair(t[:]), [0, B], [SJ, BJ], [1, BK]]
        )
        a.offset = (2 + di) * SI + 2 * SJ + 2
        return a

    def tvi(t, B, di):
        """[B, BJ, BK] view of temp tile at i-plane di (strides FD, BK, 1)."""
        a = t[:].copy()
        a.ap = bass_rust.VecI64Pair(
            [part_pair(t[:]), [FD, B], [BK, BJ], [1, BK]]
        )
        a.offset = di * BJ * BK
        return a

    # shift groups: (s0, s1) in [-2,2]^2; k-batches over s2.
    # batch m=0..B-1 corresponds to s2 = s2_hi - m, read k-offset = (2 - s2_hi) + m
    groups = []
    for s0 in range(-2, 3):
        for s1 in range(-2, 3):
            if s0 == 0 and s1 == 0:
                groups.append((0, 0, 2, 2))    # s2 in {2,1}
                groups.append((0, 0, -1, 2))   # s2 in {-1,-2}
            else:
                groups.append((s0, s1, 2, 5))  # s2 in {2..-2}

    # temp slot plan: tag -> bufs (all sized for B=5 * FD)
    TAGS = {
        "dx": 2, "dy": 2, "dz": 2, "dvx": 1, "dvy": 1, "dvz": 1,
        "q": 2, "sx": 1, "sy": 1, "sz": 1, "v": 1, "rr": 1, "cv": 1,
        "cs": 1, "bs": 1, "p1": 1, "p2": 1, "p3": 1,
    }

    with tile.TileContext(nc) as tc:
        import contextlib
        with contextlib.ExitStack() as ctx:
            ipool = ctx.enter_context(tc.tile_pool(name="in", bufs=1))
            dpool = ctx.enter_context(tc.tile_pool(name="tmp", bufs=1))
            opool = ctx.enter_context(tc.tile_pool(name="out", bufs=2))
            ppool = ctx.enter_context(tc.tile_pool(name="psum", bufs=1, space="PSUM"))
            cpool = ctx.enter_context(tc.tile_pool(name="const", bufs=1))
            spool = ctx.enter_context(tc.tile_pool(name="small", bufs=2))

            ident_f = cpool.tile([128, 128], F32, name="ident_f")
            nc.sync.dma_start(ident_f[:], ident_d.ap())
            ident_s = cpool.tile([128, 128], F32R, name="ident_s")
            nc.vector.tensor_copy(ident_s[:], ident_f[:])
            zero_f = cpool.tile([128, FD], F32, name="zero_f")
            nc.vector.memset(zero_f[:], 0.0)
            zero_r = cpool.tile([128, FD], F32R, name="zero_r")
            nc.vector.tensor_copy(zero_r[:], zero_f[:])
            ones_f = cpool.tile([128, 1], F32, name="ones_f")
            nc.vector.memset(ones_f[:], 1.0)
            OrderedSet = bass.OrderedSet
            skip_regs = [nc.alloc_registers(
                f"skipregs{i}",
                OrderedSet([mybir.EngineType.DVE, mybir.EngineType.Pool,
                            mybir.EngineType.Activation, mybir.EngineType.PE]),
            ) for i in range(2)]
            NEG_FOURD2_BITS = int(np.float32(-fourd2).view(np.int32))

            def body(r):
                tin = {}
                for nm in NAMES:
                    t = ipool.tile([128, BLKE], F32, tag=nm, name=f"in_{nm}")
                    src = dram_in[nm].ap()[bass.ds(r, 1), :, :]
                    nc.sync.dma_start(t[:].rearrange("p (u f) -> p u f", u=1), src)
                    tin[nm] = t

                accs = [ppool.tile([128, FD], F32, tag=f"acc{i}", name=f"acc{i}")
                        for i in range(6)]
                for ai in range(6):  # open accumulation groups (clear PSUM)
                    nc.tensor.matmul(accs[ai][:], ident_s[:], zero_r[:],
                                     start=True, stop=False, skip_group_check=True)

                def accum(ai, src_ap):
                    nc.tensor.matmul(accs[ai][:], ident_s[:], src_ap,
                                     start=False, stop=False, skip_group_check=True)

                V, G, A = nc.vector, nc.gpsimd, nc.scalar

                def mkt(tag, B, name, dtype=F32):
                    # allocate a [128, B*FD] temp in slot `tag` (slot sized 5*FD)
                    return dpool.tile([128, B * FD], dtype, tag=tag,
                                      bufs=TAGS[tag], name=name,
                                      padded_shape=[128, 5 * FD])

                spool_i = [0]

                def small(name):
                    spool_i[0] += 1
                    return spool.tile([128, 1], F32, tag=f"s{spool_i[0] % 4}",
                                      name=f"{name}{spool_i[0]}")

                def emit_always(gi, s0, s1, s2hi, B):
                    def tt(tag, name):
                        return mkt(tag, B, name)
                    dx = tt("dx", "dx")
                    dy = tt("dy", "dy")
                    dz = tt("dz", "dz")
                    for di in range(BI):
                        V.tensor_tensor(tvi(dx, B, di), basei(tin["x"], B, di), wini(tin["x"], s0, s1, 2 - s2hi, B, di), OP.subtract)
                    for di in range(BI):
                        V.tensor_tensor(tvi(dy, B, di), basei(tin["y"], B, di), wini(tin["y"], s0, s1, 2 - s2hi, B, di), OP.subtract)
                    for di in range(BI):
                        V.tensor_tensor(tvi(dz, B, di), basei(tin["z"], B, di), wini(tin["z"], s0, s1, 2 - s2hi, B, di), OP.subtract)
                    sx = tt("sx", "sx")
                    sy = tt("sy", "sy")
                    sz = tt("sz", "sz")
                    A.activation(sx[:], dx[:], AF.Square)
                    A.activation(sy[:], dy[:], AF.Square)
                    A.activation(sz[:], dz[:], AF.Square)
                    q = tt("q", "q")
                    G.tensor_tensor(sx[:], sx[:], sy[:], OP.add)
                    G.tensor_tensor(q[:], sx[:], sz[:], OP.add)
                    junk = mkt("cs", B, "junk")
                    msum = small("msum")
                    V.tensor_scalar(junk[:], q[:], fourd2, 0.0, OP.is_lt, OP.add,
                                    accum_out=msum[:])
                    pp = ppool.tile([1, 1], F32, tag="hit", name="hit", bufs=2)
                    nc.tensor.matmul(pp[:1, :1], ones_f[:, :1], msum[:, :1],
                                     start=True, stop=True, skip_group_check=True)
                    hits = small("hits")
                    V.tensor_copy(hits[:1, :1], pp[:1, :1])
                    regs = skip_regs[gi % 2]
                    nc.regs_load(regs, hits[:1, :1].bitcast(mybir.dt.int32))
                    return regs, dx, dy, dz, q

                def emit_hot(state, s0, s1, s2hi, B):
                    regs, dx, dy, dz, q = state
                    def tt(tag, name):
                        return mkt(tag, B, name)
                    with tc.If(nc.snap(regs) > 0):
                        l = tt("sx", "l")
                        A.activation(l[:], q[:], AF.Ln)
                        r0 = tt("sy", "r0")
                        A.activation(r0[:], l[:], AF.Exp, scale=-0.5)
                        u = tt("sz", "u")
                        A.activation(u[:], r0[:], AF.Square)
                        v = tt("v", "v")
                        V.tensor_tensor(v[:], q[:], u[:], OP.mult)
                        A.activation(v[:], v[:], AF.Copy, bias=1.5, scale=-0.5)
                        rr = tt("rr", "rr")
                        V.tensor_tensor(rr[:], r0[:], v[:], OP.mult)
                        cv = tt("cv", "cv")
                        A.activation(cv[:], rr[:], AF.Copy, bias=1.0, scale=-float(two_d))
                        cs = tt("cs", "cs")
                        V.scalar_tensor_tensor(cs[:], q[:], fourd2, cv[:], OP.is_lt, OP.mult)
                        dvx = tt("dvx", "dvx")
                        dvy = tt("dvy", "dvy")
                        dvz = tt("dvz", "dvz")
                        for di in range(BI):
                            V.tensor_tensor(tvi(dvx, B, di), basei(tin["vx"], B, di), wini(tin["vx"], s0, s1, 2 - s2hi, B, di), OP.subtract)
                        for di in range(BI):
                            V.tensor_tensor(tvi(dvy, B, di), basei(tin["vy"], B, di), wini(tin["vy"], s0, s1, 2 - s2hi, B, di), OP.subtract)
                        for di in range(BI):
                            V.tensor_tensor(tvi(dvz, B, di), basei(tin["vz"], B, di), wini(tin["vz"], s0, s1, 2 - s2hi, B, di), OP.subtract)
                        p1 = tt("p1", "p1")
                        p2 = tt("p2", "p2")
                        p3 = tt("p3", "p3")
                        V.tensor_tensor(p1[:], dvx[:], dx[:], OP.mult)
                        V.tensor_tensor(p2[:], dvy[:], dy[:], OP.mult)
                        G.tensor_tensor(p3[:], dvz[:], dz[:], OP.mult)
                        G.tensor_tensor(p1[:], p1[:], p2[:], OP.add)
                        G.tensor_tensor(p1[:], p1[:], p3[:], OP.add)
                        r2 = tt("sx", "r2")
                        A.activation(r2[:], rr[:], AF.Square)
                        wd = tt("cv", "wd")
                        V.tensor_tensor(wd[:], p1[:], r2[:], OP.mult)
                        bs = tt("bs", "bs")
                        V.scalar_tensor_tensor(bs[:], q[:], fourd2, wd[:], OP.is_lt, OP.mult)
                        cx = mkt("dvx", B, "cx", F32R)
                        cy = mkt("dvy", B, "cy", F32R)
                        cz = mkt("dvz", B, "cz", F32R)
                        ex = mkt("p2", B, "ex", F32R)
                        ey = mkt("p3", B, "ey", F32R)
                        ez = mkt("sz", B, "ez", F32R)
                        V.tensor_tensor(cx[:], cs[:], dx[:], OP.mult)
                        V.tensor_tensor(cy[:], cs[:], dy[:], OP.mult)
                        V.tensor_tensor(cz[:], cs[:], dz[:], OP.mult)
                        G.tensor_tensor(ex[:], bs[:], dx[:], OP.mult)
                        G.tensor_tensor(ey[:], bs[:], dy[:], OP.mult)
                        G.tensor_tensor(ez[:], bs[:], dz[:], OP.mult)
                        for m in range(B):
                            sl = slice(m * FD, (m + 1) * FD)
                            accum(0, cx[:, sl])
                            accum(1, cy[:, sl])
                            accum(2, cz[:, sl])
                            accum(3, ex[:, sl])
                            accum(4, ey[:, sl])
                            accum(5, ez[:, sl])

                pending = None
                for gi, (s0, s1, s2hi, B) in enumerate(groups):
                    state = emit_always(gi, s0, s1, s2hi, B)
                    if pending is not None:
                        emit_hot(*pending)
                    pending = (state, s0, s1, s2hi, B)
                emit_hot(*pending)

                for ai in range(6):  # close accumulation groups
                    nc.tensor.matmul(accs[ai][:], ident_s[:], zero_r[:],
                                     start=False, stop=True, skip_group_check=True)

                # ---- scale accumulators to output force values ----
                outs = []
                for o in range(6):
                    ot = opool.tile([128, FD], F32, tag=f"o{o}", name=f"o{o}")
                    A.activation(ot[:], accs[o][:], AF.Copy, scale=(KN if o < 3 else ETA))
                    outs.append(ot)

                # ---- friction from last shift (2,2,2) ----
                # temps reuse the main group slots (all groups are done here)
                dxl = mkt("dx", 1, "dxl")
                dyl = mkt("dy", 1, "dyl")
                dzl = mkt("dz", 1, "dzl")
                for di in range(BI):
                    V.tensor_tensor(tvi(dxl, 1, di), basei(tin["x"], 1, di), wini(tin["x"], 2, 2, 0, 1, di), OP.subtract)
                for di in range(BI):
                    V.tensor_tensor(tvi(dyl, 1, di), basei(tin["y"], 1, di), wini(tin["y"], 2, 2, 0, 1, di), OP.subtract)
                for di in range(BI):
                    V.tensor_tensor(tvi(dzl, 1, di), basei(tin["z"], 1, di), wini(tin["z"], 2, 2, 0, 1, di), OP.subtract)
                t1 = mkt("sx", 1, "t1")
                t2 = mkt("sy", 1, "t2")
                ql = mkt("q", 1, "ql")
                A.activation(t1[:], dxl[:], AF.Square)
                A.activation(t2[:], dyl[:], AF.Square)
                A.activation(ql[:], dzl[:], AF.Square)
                G.tensor_tensor(t1[:], t1[:], t2[:], OP.add)
                G.tensor_tensor(ql[:], t1[:], ql[:], OP.add)
                mneg = mkt("cs", 1, "mneg")  # -(q < (2d)^2)
                V.tensor_scalar(mneg[:], ql[:], fourd2, -1.0, OP.is_lt, OP.mult)
                dvxl = mkt("dvx", 1, "dvxl")
                dvyl = mkt("dvy", 1, "dvyl")
                dvzl = mkt("dvz", 1, "dvzl")
                for di in range(BI):
                    V.tensor_tensor(tvi(dvxl, 1, di), basei(tin["vx"], 1, di), wini(tin["vx"], 2, 2, 0, 1, di), OP.subtract)
                for di in range(BI):
                    V.tensor_tensor(tvi(dvyl, 1, di), basei(tin["vy"], 1, di), wini(tin["vy"], 2, 2, 0, 1, di), OP.subtract)
                for di in range(BI):
                    V.tensor_tensor(tvi(dvzl, 1, di), basei(tin["vz"], 1, di), wini(tin["vz"], 2, 2, 0, 1, di), OP.subtract)

                def safe_recip_abs(dv, rtag, idx):
                    # 1 / max(EPS, |dv|), exp/ln + 2 Newton steps
                    aa = mkt("sz", 1, f"aa{idx}")
                    A.activation(aa[:], dv[:], AF.Abs)
                    V.tensor_scalar(aa[:], aa[:], EPS, None, OP.max)
                    ll = mkt("v", 1, f"ll{idx}")
                    A.activation(ll[:], aa[:], AF.Ln)
                    rr0 = mkt(rtag, 1, f"rcp{idx}")
                    A.activation(rr0[:], ll[:], AF.Exp, scale=-1.0)
                    for it in range(2):
                        tn = mkt("cv", 1, f"tn{idx}_{it}")
                        V.tensor_tensor(tn[:], aa[:], rr0[:], OP.mult)
                        V.tensor_scalar(tn[:], tn[:], -1.0, 2.0, OP.mult, OP.add)
                        V.tensor_tensor(rr0[:], rr0[:], tn[:], OP.mult)
                    return rr0

                rx = safe_recip_abs(dvxl, "p1", 0)
                ry = safe_recip_abs(dvyl, "p2", 1)
                rz = safe_recip_abs(dvzl, "p3", 2)
                # numerator factors: dvx/|dvx|_safe, dvy/|dvy|_safe, dvy/|dvz|_safe
                fax = mkt("rr", 1, "fax")
                fay = mkt("bs", 1, "fay")
                faz = mkt("dx", 1, "faz")
                V.tensor_tensor(fax[:], dvxl[:], rx[:], OP.mult)
                V.tensor_tensor(fay[:], dvyl[:], ry[:], OP.mult)
                V.tensor_tensor(faz[:], dvyl[:], rz[:], OP.mult)

                # |f_c| of scaled collision forces
                afx = mkt("dy", 1, "afx")
                afy = mkt("dz", 1, "afy")
                afz = mkt("sx", 1, "afz")
                A.activation(afx[:], outs[0][:], AF.Abs)
                A.activation(afy[:], outs[1][:], AF.Abs)
                A.activation(afz[:], outs[2][:], AF.Abs)

                fr_defs = [
                    (afy, afz, outs[3], fax),  # frx: |fyc|+|fzc| - fxd, dir x
                    (afx, afz, outs[4], fay),  # fry: |fxc|+|fzc| - fyd, dir y
                    (afx, afy, outs[5], faz),  # frz: |fxc|+|fyc| - fzd, dir (vy/|vz|)
                ]
                for o, (a1, a2, dmp, fac) in enumerate(fr_defs):
                    u1 = mkt("sy", 1, f"u1_{o}")
                    G.tensor_tensor(u1[:], a1[:], a2[:], OP.add)
                    G.tensor_tensor(u1[:], u1[:], dmp[:], OP.subtract)
                    u2 = mkt("q", 1, f"u2_{o}")
                    A.activation(u2[:], u1[:], AF.Abs, scale=MU)
                    V.tensor_tensor(u2[:], u2[:], fac[:], OP.mult)
                    ot = opool.tile([128, FD], F32, tag=f"o{6+o}", name=f"o{6+o}")
                    V.tensor_tensor(ot[:], u2[:], mneg[:], OP.mult)
                    outs.append(ot)

                for o in range(9):
                    dst = out_d.ap()[bass.ds(r, 1), o, :, :]
                    nc.sync.dma_start(dst, outs[o][:].rearrange("p (u f) -> p u f", u=1))

            with tc.For_i(0, ROUNDS, 1) as r:
                body(r)

    return nc


# ---------------- host-side sharding ----------------

def _split_excess_waits(nc, max_waits=1):
    """This walrus build allows only max_waits semaphore waits per instruction;
    hoist the excess onto NoOps inserted just before the offender."""
    from concourse import mybir
    cnt = 0
    for blk in nc.m.functions[0].blocks:
        new = []
        changed = False
        for ins in blk.instructions:
            si = ins.sync_info
            if si is not None and si.on_wait and len(si.on_wait) > max_waits:
                waits = list(si.on_wait)
                keep = waits[-max_waits:]
                extra = waits[:-max_waits]
                for i in range(0, len(extra), max_waits):
                    nop = mybir.InstNoOp(name=f"wait_split_{cnt}", ins=[], outs=[])
                    cnt += 1
                    nop.engine = ins.engine
                    nop.sync_info = type(si)(on_wait=extra[i:i + max_waits], on_update=[])
                    new.append(nop)
                ins.sync_info = type(si)(on_wait=keep, on_update=si.on_update)
                changed = True
            new.append(ins)
        if changed:
            blk.instructions = new
    return nc


def _prep_in_maps(inputs):
    from numpy.lib.stride_tricks import sliding_window_view

    arrs = {
        "x": inputs["x_grid"], "y": inputs["y_grid"], "z": inputs["z_grid"],
        "vx": inputs["vx_grid"], "vy": inputs["vy_grid"], "vz": inputs["vz_grid"],
    }
    lanes = np.arange(128)
    j_starts = (lanes % JBLK) * BJ                 # 0..180
    k_starts = np.arange(ROUNDS) * BK              # 0..184
    ident = np.eye(128, dtype=np.float32)

    in_maps = [dict() for _ in range(CORES)]
    for nm, a in arrs.items():
        ap = np.pad(np.asarray(a, dtype=np.float32), 2, mode="wrap")  # [196]^3
        W = sliding_window_view(ap, (HI, HJ, HK))  # [190,181,185,7,16,12] view
        for c in range(CORES):
            i_starts = c * NIP + (lanes // JBLK) * BI
            blk = W[i_starts[None, :], j_starts[None, :], k_starts[:, None]]
            in_maps[c][nm] = np.ascontiguousarray(
                blk.reshape(ROUNDS, 128, BLKE), dtype=np.float32)
    for c in range(CORES):
        in_maps[c]["ident"] = ident
    return in_maps


def _gather(results):
    out = np.empty((9, N, N, N), dtype=np.float32)
    for c in range(CORES):
        r = results[c]["out"].reshape(ROUNDS, 9, IBLK, JBLK, BI, BJ, BK)
        # [r, o, ib, jb, di, dj, dk] -> [o, ib, di, jb, dj, r, dk]
        blk = r.transpose(1, 2, 4, 3, 5, 0, 6).reshape(9, NIP, N, N)
        out[:, c * NIP:(c + 1) * NIP] = blk
    return out


def _run(inputs, trace=False):
    from concourse.bass_utils import run_bass_kernel_spmd

    d = float(np.asarray(inputs["d"]))
    two_d = 2.0 * d
    key = round(two_d, 9)
    if key not in _cache:
        _cache[key] = _split_excess_waits(_build_nc(two_d))
    nc = _cache[key]
    in_maps = _prep_in_maps(inputs)
    res = run_bass_kernel_spmd(nc, in_maps, core_ids=list(range(CORES)), trace=trace)
    return _gather(res.results), res


def kernel(**inputs):
    out, _ = _run(inputs, trace=False)
    return out


# revision 22
# speedup vs baseline: 2.8433x; 1.0368x over previous
"""DEM particle-force stencil (125-shift 5x5x5 neighborhood) on 8 TRN2 NeuronCores.

Self-contained: hardcodes shapes/sharding. kernel(**inputs) takes full-size
numpy arrays, shards across 8 cores with halos baked host-side, runs a Bass
kernel per core (SPMD, no collectives), and reassembles the full output.

Strategy: 128 lanes own [3,12,12] interior blocks (halos in the free dim);
16 k-rounds via For_i; per 5-shift batch an any-overlap test (reduce + PE
ones-matmul) gates the expensive force math behind tc.If (hits are ~1e-6
sparse); accumulation via float32r identity matmuls into PSUM.
"""
import math
import numpy as np

# ---------------- problem constants ----------------
N = 192
CORES = 8
KN = 500000.0
MU = 0.5
EPS = 1e-4
_alpha = -math.log(0.7) / math.pi
_gamma = _alpha / math.sqrt(_alpha**2 + 1.0)
ETA = 2.0 * _gamma * math.sqrt(KN * 1.0)

# ---------------- layout constants ----------------
BI, BJ, BK = 3, 12, 12         # interior block per lane
IBLK, JBLK = 8, 16             # 8*16 = 128 lanes; covers 24 x 192 (i x j) per core
NIP = N // CORES               # 24 i-planes per core
ROUNDS = N // BK               # 16 k-rounds
FD = BI * BJ * BK              # 432 interior elems per lane per round
HI, HJ, HK = BI + 4, BJ + 4, BK + 4   # 7,16,16 haloed block
BLKE = HI * HJ * HK            # 1792
SI, SJ = HJ * HK, HK           # free-dim strides of haloed block: 256, 16
NAMES = ["x", "y", "z", "vx", "vy", "vz"]

_cache = {}


def _build_nc(two_d):
    import concourse.bass as bass
    import concourse.tile as tile
    import bass_rust
    from concourse import mybir

    F32 = mybir.dt.float32
    F32R = mybir.dt.float32r
    AF = mybir.ActivationFunctionType
    OP = mybir.AluOpType
    fourd2 = float(two_d * two_d)

    nc = bass.Bass(target_bir_lowering=False, debug=False)
    dram_in = {
        nm: nc.declare_dram_parameter(nm, [ROUNDS, 128, BLKE], F32, isOutput=False)
        for nm in NAMES
    }
    ident_d = nc.declare_dram_parameter("ident", [128, 128], F32, isOutput=False)
    out_d = nc.declare_dram_parameter("out", [ROUNDS, 9, 128, FD], F32, isOutput=True)

    def part_pair(ap):
        return list(ap.ap.to_list()[0])

    def wini(t, s0, s1, koff, B, di):
        a = t[:].copy()
        a.ap = bass_rust.VecI64Pair(
            [part_pair(t[:]), [1, B], [SJ, BJ], [1, BK]]
        )
        a.offset = (2 - s0 + di) * SI + (2 - s1) * SJ + koff
        return a

    def basei(t, B, di):
        a = t[:].copy()
        a.ap = bass_rust.VecI64Pair(
            [part_pair(t[:]), [0, B], [SJ, BJ], [1, BK]]
        )
        a.offset = (2 + di) * SI + 2 * SJ + 2
        return a

    def tvi(t, B, di):
        a = t[:].copy()
        a.ap = bass_rust.VecI64Pair(
            [part_pair(t[:]), [FD, B], [BK, BJ], [1, BK]]
        )
        a.offset = di * BJ * BK
        return a

    groups = []
    for s0 in range(-2, 3):
        for s1 in range(-2, 3):
            if s0 == 0 and s1 == 0:
                groups.append((0, 0, 2, 2))
                groups.append((0, 0, -1, 2))
            else:
                groups.append((s0, s1, 2, 5))

    # Tiles read inside If_g but re-allocated by the pipelined always-part of
    # g+1 (emitted before If_g) need bufs=2.
    TAGS = {
        "dx": 2, "dy": 2, "dz": 2, "q": 2,
        "dvx": 1, "dvy": 1, "dvz": 1,
        "sx": 1, "sy": 1, "v": 1, "rr": 1, "cs": 1,
    }

    with tile.TileContext(nc) as tc:
        import contextlib
        with contextlib.ExitStack() as ctx:
            ipool = ctx.enter_context(tc.tile_pool(name="in", bufs=1))
            dpool = ctx.enter_context(tc.tile_pool(name="tmp", bufs=1))
            opool = ctx.enter_context(tc.tile_pool(name="out", bufs=1))
            ppool = ctx.enter_context(tc.tile_pool(name="psum", bufs=1, space="PSUM"))
            cpool = ctx.enter_context(tc.tile_pool(name="const", bufs=1))
            spool = ctx.enter_context(tc.tile_pool(name="small", bufs=2))

            ident_f = cpool.tile([128, 128], F32, name="ident_f")
            nc.sync.dma_start(ident_f[:], ident_d.ap())
            ident_s = cpool.tile([128, 128], F32R, name="ident_s")
            nc.vector.tensor_copy(ident_s[:], ident_f[:])
            zero_f = dpool.tile([128, FD], F32, tag="v", bufs=1,
                                name="zero_f", padded_shape=[128, 5 * FD])
            nc.vector.memset(zero_f[:], 0.0)
            zero_r = cpool.tile([128, FD], F32R, name="zero_r")
            nc.vector.tensor_copy(zero_r[:], zero_f[:])
            ones_f = cpool.tile([128, 1], F32, name="ones_f")
            nc.vector.memset(ones_f[:], 1.0)
            OrderedSet = bass.OrderedSet
            skip_regs = [nc.alloc_registers(
                f"skipregs{i}",
                OrderedSet([mybir.EngineType.DVE,
                            mybir.EngineType.Activation, mybir.EngineType.PE]),
            ) for i in range(2)]

            def body(r):
                tin = {}
                for nm in NAMES:
                    t = ipool.tile([128, BLKE], F32, tag=nm, name=f"in_{nm}")
                    src = dram_in[nm].ap()[bass.ds(r, 1), :, :]
                    nc.sync.dma_start(t[:].rearrange("p (u f) -> p u f", u=1), src)
                    tin[nm] = t

                accs = [ppool.tile([128, FD], F32, tag=f"acc{i}", name=f"acc{i}")
                        for i in range(6)]
                for ai in range(6):  # open accumulation groups (clear PSUM banks)
                    nc.tensor.matmul(accs[ai][:], ident_s[:], zero_r[:],
                                     start=True, stop=False, skip_group_check=True)

                def accum(ai, src_ap):
                    nc.tensor.matmul(accs[ai][:], ident_s[:], src_ap,
                                     start=False, stop=False, skip_group_check=True)

                V, G, A = nc.vector, nc.gpsimd, nc.scalar

                def mkt(tag, B, name, dtype=F32):
                    return dpool.tile([128, B * FD], dtype, tag=tag,
                                      bufs=TAGS[tag], name=name,
                                      padded_shape=[128, 5 * FD])

                spool_i = [0]

                def small(name):
                    spool_i[0] += 1
                    return spool.tile([128, 1], F32, tag=f"s{spool_i[0] % 4}",
                                      name=f"{name}{spool_i[0]}")

                def emit_always(gi, s0, s1, s2hi, B):
                    dx = mkt("dx", B, "dx")
                    dy = mkt("dy", B, "dy")
                    dz = mkt("dz", B, "dz")
                    for di in range(BI):
                        V.tensor_tensor(tvi(dx, B, di), basei(tin["x"], B, di), wini(tin["x"], s0, s1, 2 - s2hi, B, di), OP.subtract)
                    for di in range(BI):
                        V.tensor_tensor(tvi(dy, B, di), basei(tin["y"], B, di), wini(tin["y"], s0, s1, 2 - s2hi, B, di), OP.subtract)
                    for di in range(BI):
                        V.tensor_tensor(tvi(dz, B, di), basei(tin["z"], B, di), wini(tin["z"], s0, s1, 2 - s2hi, B, di), OP.subtract)
                    sx = mkt("sx", B, "sx")
                    sy = mkt("sy", B, "sy")
                    q = mkt("q", B, "q")
                    A.activation(sx[:], dx[:], AF.Square)
                    A.activation(sy[:], dy[:], AF.Square)
                    A.activation(q[:], dz[:], AF.Square)
                    G.tensor_tensor(sx[:], sx[:], sy[:], OP.add)
                    G.tensor_tensor(q[:], q[:], sx[:], OP.add)
                    junk = mkt("cs", B, "junk")
                    msum = small("msum")
                    V.tensor_scalar(junk[:], q[:], fourd2, 0.0, OP.is_lt, OP.add,
                                    accum_out=msum[:])
                    pp = ppool.tile([1, 1], F32, tag="hit", name="hit", bufs=2)
                    nc.tensor.matmul(pp[:1, :1], ones_f[:, :1], msum[:, :1],
                                     start=True, stop=True, skip_group_check=True)
                    hits = small("hits")
                    V.tensor_copy(hits[:1, :1], pp[:1, :1])
                    regs = skip_regs[gi % 2]
                    nc.regs_load(regs, hits[:1, :1].bitcast(mybir.dt.int32))
                    return regs, dx, dy, dz, q

                def emit_hot(state, s0, s1, s2hi, B):
                    regs, dx, dy, dz, q = state
                    with tc.If(nc.snap(regs) > 0):
                        l = mkt("sx", B, "l")
                        A.activation(l[:], q[:], AF.Ln)
                        r0 = mkt("sy", B, "r0")
                        A.activation(r0[:], l[:], AF.Exp, scale=-0.5)
                        u = mkt("cs", B, "u")
                        A.activation(u[:], r0[:], AF.Square)
                        v = mkt("v", B, "v")
                        V.tensor_tensor(v[:], q[:], u[:], OP.mult)
                        A.activation(v[:], v[:], AF.Copy, bias=1.5, scale=-0.5)
                        rr = mkt("rr", B, "rr")
                        V.tensor_tensor(rr[:], r0[:], v[:], OP.mult)
                        cv = mkt("sy", B, "cv")
                        A.activation(cv[:], rr[:], AF.Copy, bias=1.0, scale=-float(two_d))
                        cs = mkt("cs", B, "cs")
                        V.scalar_tensor_tensor(cs[:], q[:], fourd2, cv[:], OP.is_lt, OP.mult)
                        dvx = mkt("dvx", B, "dvx")
                        dvy = mkt("dvy", B, "dvy")
                        dvz = mkt("dvz", B, "dvz")
                        for di in range(BI):
                            V.tensor_tensor(tvi(dvx, B, di), basei(tin["vx"], B, di), wini(tin["vx"], s0, s1, 2 - s2hi, B, di), OP.subtract)
                        for di in range(BI):
                            V.tensor_tensor(tvi(dvy, B, di), basei(tin["vy"], B, di), wini(tin["vy"], s0, s1, 2 - s2hi, B, di), OP.subtract)
                        for di in range(BI):
                            V.tensor_tensor(tvi(dvz, B, di), basei(tin["vz"], B, di), wini(tin["vz"], s0, s1, 2 - s2hi, B, di), OP.subtract)
                        p1 = mkt("v", B, "p1")
                        p2 = mkt("sy", B, "p2")
                        p3 = mkt("sx", B, "p3")
                        V.tensor_tensor(p1[:], dvx[:], dx[:], OP.mult)
                        V.tensor_tensor(p2[:], dvy[:], dy[:], OP.mult)
                        V.tensor_tensor(p3[:], dvz[:], dz[:], OP.mult)
                        V.tensor_tensor(p1[:], p1[:], p2[:], OP.add)
                        V.tensor_tensor(p1[:], p1[:], p3[:], OP.add)
                        r2 = mkt("sx", B, "r2")
                        A.activation(r2[:], rr[:], AF.Square)
                        wd = mkt("sy", B, "wd")
                        V.tensor_tensor(wd[:], p1[:], r2[:], OP.mult)
                        bs = mkt("dvx", B, "bs")
                        V.scalar_tensor_tensor(bs[:], q[:], fourd2, wd[:], OP.is_lt, OP.mult)
                        # damping fields first (frees bs slot), then collision
                        ex = mkt("rr", B, "ex", F32R)
                        ey = mkt("sy", B, "ey", F32R)
                        ez = mkt("sx", B, "ez", F32R)
                        V.tensor_tensor(ex[:], bs[:], dx[:], OP.mult)
                        V.tensor_tensor(ey[:], bs[:], dy[:], OP.mult)
                        V.tensor_tensor(ez[:], bs[:], dz[:], OP.mult)
                        cx = mkt("dvx", B, "cx", F32R)
                        cy = mkt("dvy", B, "cy", F32R)
                        cz = mkt("dvz", B, "cz", F32R)
                        V.tensor_tensor(cx[:], cs[:], dx[:], OP.mult)
                        V.tensor_tensor(cy[:], cs[:], dy[:], OP.mult)
                        V.tensor_tensor(cz[:], cs[:], dz[:], OP.mult)
                        for m in range(B):
                            sl = slice(m * FD, (m + 1) * FD)
                            accum(3, ex[:, sl])
                            accum(4, ey[:, sl])
                            accum(5, ez[:, sl])
                            accum(0, cx[:, sl])
                            accum(1, cy[:, sl])
                            accum(2, cz[:, sl])

                pending = None
                for gi, (s0, s1, s2hi, B) in enumerate(groups):
                    state = emit_always(gi, s0, s1, s2hi, B)
                    if pending is not None:
                        emit_hot(*pending)
                    pending = (state, s0, s1, s2hi, B)
                emit_hot(*pending)

                for ai in range(6):  # close accumulation groups
                    nc.tensor.matmul(accs[ai][:], ident_s[:], zero_r[:],
                                     start=False, stop=True, skip_group_check=True)

                # ---- scale accumulators to output force values ----
                outs = []
                for o in range(6):
                    ot = opool.tile([128, FD], F32, tag=f"o{o}", name=f"o{o}")
                    A.activation(ot[:], accs[o][:], AF.Copy, scale=(KN if o < 3 else ETA))
                    outs.append(ot)

                # ---- friction from last shift (2,2,2), gated on last-shift hits ----
                dxl = mkt("dx", 1, "dxl")
                dyl = mkt("dy", 1, "dyl")
                dzl = mkt("dz", 1, "dzl")
                for di in range(BI):
                    V.tensor_tensor(tvi(dxl, 1, di), basei(tin["x"], 1, di), wini(tin["x"], 2, 2, 0, 1, di), OP.subtract)
                for di in range(BI):
                    V.tensor_tensor(tvi(dyl, 1, di), basei(tin["y"], 1, di), wini(tin["y"], 2, 2, 0, 1, di), OP.subtract)
                for di in range(BI):
                    V.tensor_tensor(tvi(dzl, 1, di), basei(tin["z"], 1, di), wini(tin["z"], 2, 2, 0, 1, di), OP.subtract)
                t1 = mkt("sx", 1, "t1")
                t2 = mkt("sy", 1, "t2")
                ql = mkt("q", 1, "ql")
                A.activation(t1[:], dxl[:], AF.Square)
                A.activation(t2[:], dyl[:], AF.Square)
                A.activation(ql[:], dzl[:], AF.Square)
                G.tensor_tensor(t1[:], t1[:], t2[:], OP.add)
                G.tensor_tensor(ql[:], ql[:], t1[:], OP.add)
                junkl = mkt("cs", 1, "junkl")
                msl = small("msl")
                V.tensor_scalar(junkl[:], ql[:], fourd2, 0.0, OP.is_lt, OP.add,
                                accum_out=msl[:])
                ppl = ppool.tile([1, 1], F32, tag="hit", name="hitl", bufs=2)
                nc.tensor.matmul(ppl[:1, :1], ones_f[:, :1], msl[:, :1],
                                 start=True, stop=True, skip_group_check=True)
                hitsl = small("hitsl")
                V.tensor_copy(hitsl[:1, :1], ppl[:1, :1])
                nc.regs_load(skip_regs[0], hitsl[:1, :1].bitcast(mybir.dt.int32))

                fr_outs = []
                for o in range(3):
                    ot = opool.tile([128, FD], F32, tag=f"o{6+o}", name=f"o{6+o}")
                    V.memset(ot[:], 0.0)
                    fr_outs.append(ot)

                with tc.If(nc.snap(skip_regs[0]) > 0):
                    mneg = mkt("cs", 1, "mneg")  # -(q < (2d)^2)
                    V.tensor_scalar(mneg[:], ql[:], fourd2, -1.0, OP.is_lt, OP.mult)
                    dvxl = mkt("dvx", 1, "dvxl")
                    dvyl = mkt("dvy", 1, "dvyl")
                    dvzl = mkt("dvz", 1, "dvzl")
                    for di in range(BI):
                        V.tensor_tensor(tvi(dvxl, 1, di), basei(tin["vx"], 1, di), wini(tin["vx"], 2, 2, 0, 1, di), OP.subtract)
                    for di in range(BI):
                        V.tensor_tensor(tvi(dvyl, 1, di), basei(tin["vy"], 1, di), wini(tin["vy"], 2, 2, 0, 1, di), OP.subtract)
                    for di in range(BI):
                        V.tensor_tensor(tvi(dvzl, 1, di), basei(tin["vz"], 1, di), wini(tin["vz"], 2, 2, 0, 1, di), OP.subtract)

                    def safe_recip_abs(dv, rtag, idx):
                        # 1 / max(EPS, |dv|), exp/ln + 2 Newton steps
                        aa = mkt("v", 1, f"aa{idx}")
                        A.activation(aa[:], dv[:], AF.Abs)
                        V.tensor_scalar(aa[:], aa[:], EPS, None, OP.max)
                        ll = mkt("q", 1, f"ll{idx}")
                        A.activation(ll[:], aa[:], AF.Ln)
                        rr0 = mkt(rtag, 1, f"rcp{idx}")
                        A.activation(rr0[:], ll[:], AF.Exp, scale=-1.0)
                        for it in range(2):
                            tn = mkt("q", 1, f"tn{idx}_{it}")
                            V.tensor_tensor(tn[:], aa[:], rr0[:], OP.mult)
                            V.tensor_scalar(tn[:], tn[:], -1.0, 2.0, OP.mult, OP.add)
                            V.tensor_tensor(rr0[:], rr0[:], tn[:], OP.mult)
                        return rr0

                    rx = safe_recip_abs(dvxl, "sx", 0)
                    ry = safe_recip_abs(dvyl, "sy", 1)
                    rz = safe_recip_abs(dvzl, "rr", 2)
                    # numerators: dvx/|dvx|_safe, dvy/|dvy|_safe, dvy/|dvz|_safe
                    fax = mkt("dx", 1, "fax")
                    fay = mkt("dy", 1, "fay")
                    faz = mkt("dz", 1, "faz")
                    V.tensor_tensor(fax[:], dvxl[:], rx[:], OP.mult)
                    V.tensor_tensor(fay[:], dvyl[:], ry[:], OP.mult)
                    V.tensor_tensor(faz[:], dvyl[:], rz[:], OP.mult)

                    afx = mkt("dvx", 1, "afx")
                    afy = mkt("dvy", 1, "afy")
                    afz = mkt("dvz", 1, "afz")
                    A.activation(afx[:], outs[0][:], AF.Abs)
                    A.activation(afy[:], outs[1][:], AF.Abs)
                    A.activation(afz[:], outs[2][:], AF.Abs)

                    fr_defs = [
                        (afy, afz, outs[3], fax),
                        (afx, afz, outs[4], fay),
                        (afx, afy, outs[5], faz),
                    ]
                    for o, (a1, a2, dmp, fac) in enumerate(fr_defs):
                        u1 = mkt("v", 1, f"u1_{o}")
                        V.tensor_tensor(u1[:], a1[:], a2[:], OP.add)
                        V.tensor_tensor(u1[:], u1[:], dmp[:], OP.subtract)
                        u2 = mkt("q", 1, f"u2_{o}")
                        A.activation(u2[:], u1[:], AF.Abs, scale=MU)
                        V.tensor_tensor(u2[:], u2[:], fac[:], OP.mult)
                        V.tensor_tensor(fr_outs[o][:], u2[:], mneg[:], OP.mult)
                outs.extend(fr_outs)

                for o in range(9):
                    dst = out_d.ap()[bass.ds(r, 1), o, :, :]
                    nc.sync.dma_start(dst, outs[o][:].rearrange("p (u f) -> p u f", u=1))

            with tc.For_i(0, ROUNDS, 1) as r:
                body(r)

    return nc


def _split_excess_waits(nc, max_waits=1):
    """This walrus build allows only max_waits semaphore waits per instruction;
    hoist the excess onto NoOps inserted just before the offender."""
    from concourse import mybir
    cnt = 0
    for blk in nc.m.functions[0].blocks:
        new = []
        changed = False
        for ins in blk.instructions:
            si = ins.sync_info
            if si is not None and si.on_wait and len(si.on_wait) > max_waits:
                waits = list(si.on_wait)
                keep = waits[-max_waits:]
                extra = waits[:-max_waits]
                for i in range(0, len(extra), max_waits):
                    nop = mybir.InstNoOp(name=f"wait_split_{cnt}", ins=[], outs=[])
                    cnt += 1
                    nop.engine = ins.engine
                    nop.sync_info = type(si)(on_wait=extra[i:i + max_waits], on_update=[])
                    new.append(nop)
                ins.sync_info = type(si)(on_wait=keep, on_update=si.on_update)
                changed = True
            new.append(ins)
        if changed:
            blk.instructions = new
    return nc


# ---------------- host-side sharding ----------------

def _prep_in_maps(inputs):
    from numpy.lib.stride_tricks import sliding_window_view

    arrs = {
        "x": inputs["x_grid"], "y": inputs["y_grid"], "z": inputs["z_grid"],
        "vx": inputs["vx_grid"], "vy": inputs["vy_grid"], "vz": inputs["vz_grid"],
    }
    lanes = np.arange(128)
    j_starts = (lanes % JBLK) * BJ
    k_starts = np.arange(ROUNDS) * BK
    ident = np.eye(128, dtype=np.float32)

    in_maps = [dict() for _ in range(CORES)]
    for nm, a in arrs.items():
        ap = np.pad(np.asarray(a, dtype=np.float32), 2, mode="wrap")  # [196]^3
        W = sliding_window_view(ap, (HI, HJ, HK))
        for c in range(CORES):
            i_starts = c * NIP + (lanes // JBLK) * BI
            blk = W[i_starts[None, :], j_starts[None, :], k_starts[:, None]]
            in_maps[c][nm] = np.ascontiguousarray(
                blk.reshape(ROUNDS, 128, BLKE), dtype=np.float32)
    for c in range(CORES):
        in_maps[c]["ident"] = ident
    return in_maps


def _gather(results):
    out = np.empty((9, N, N, N), dtype=np.float32)
    for c in range(CORES):
        r = results[c]["out"].reshape(ROUNDS, 9, IBLK, JBLK, BI, BJ, BK)
        # [r, o, ib, jb, di, dj, dk] -> [o, ib, di, jb, dj, r, dk]
        blk = r.transpose(1, 2, 4, 3, 5, 0, 6).reshape(9, NIP, N, N)
        out[:, c * NIP:(c + 1) * NIP] = blk
    return out


def _run(inputs, trace=False):
    from concourse.bass_utils import run_bass_kernel_spmd

    d = float(np.asarray(inputs["d"]))
    two_d = 2.0 * d
    key = round(two_d, 9)
    if key not in _cache:
        _cache[key] = _split_excess_waits(_build_nc(two_d))
    nc = _cache[key]
    in_maps = _prep_in_maps(inputs)
    res = run_bass_kernel_spmd(nc, in_maps, core_ids=list(range(CORES)), trace=trace)
    return _gather(res.results), res


def kernel(**inputs):
    out, _ = _run(inputs, trace=False)
    return out


# revision 25
# speedup vs baseline: 3.1207x; 1.0976x over previous
"""DEM particle-force stencil (125-shift 5x5x5 neighborhood) on 8 TRN2 NeuronCores.

Self-contained: hardcodes shapes/sharding. kernel(**inputs) takes full-size
numpy arrays, shards across 8 cores with halos baked host-side, runs a Bass
kernel per core (SPMD, no collectives), and reassembles the full output.

Strategy: 128 lanes own [3,12,12] interior blocks (halos in the free dim);
16 k-rounds via For_i; per 5-shift batch an any-overlap test (reduce + PE
ones-matmul) gates the expensive force math behind tc.If (hits are ~1e-6
sparse); accumulation via float32r identity matmuls into PSUM.
"""
import math
import numpy as np

# ---------------- problem constants ----------------
N = 192
CORES = 8
KN = 500000.0
MU = 0.5
EPS = 1e-4
_alpha = -math.log(0.7) / math.pi
_gamma = _alpha / math.sqrt(_alpha**2 + 1.0)
ETA = 2.0 * _gamma * math.sqrt(KN * 1.0)

# ---------------- layout constants ----------------
BI, BJ, BK = 3, 12, 12         # interior block per lane
IBLK, JBLK = 8, 16             # 8*16 = 128 lanes; covers 24 x 192 (i x j) per core
NIP = N // CORES               # 24 i-planes per core
ROUNDS = N // BK               # 16 k-rounds
FD = BI * BJ * BK              # 432 interior elems per lane per round
HI, HJ, HK = BI + 4, BJ + 4, BK + 4   # 7,16,16 haloed block
BLKE = HI * HJ * HK            # 1792
SI, SJ = HJ * HK, HK           # free-dim strides of haloed block: 256, 16
NAMES = ["x", "y", "z", "vx", "vy", "vz"]

_cache = {}


def _build_nc(two_d):
    import concourse.bass as bass
    import concourse.tile as tile
    import bass_rust
    from concourse import mybir

    F32 = mybir.dt.float32
    F32R = mybir.dt.float32r
    AF = mybir.ActivationFunctionType
    OP = mybir.AluOpType
    fourd2 = float(two_d * two_d)

    nc = bass.Bass(target_bir_lowering=False, debug=False)
    dram_in = {
        nm: nc.declare_dram_parameter(nm, [ROUNDS, 128, BLKE], F32, isOutput=False)
        for nm in NAMES
    }
    ident_d = nc.declare_dram_parameter("ident", [128, 128], F32, isOutput=False)
    out_d = nc.declare_dram_parameter("out", [ROUNDS, 9, 128, FD], F32, isOutput=True)

    def part_pair(ap):
        return list(ap.ap.to_list()[0])

    def wini(t, s0, s1, koff, B, di):
        a = t[:].copy()
        a.ap = bass_rust.VecI64Pair(
            [part_pair(t[:]), [1, B], [SJ, BJ], [1, BK]]
        )
        a.offset = (2 - s0 + di) * SI + (2 - s1) * SJ + koff
        return a

    def basei(t, B, di):
        a = t[:].copy()
        a.ap = bass_rust.VecI64Pair(
            [part_pair(t[:]), [0, B], [SJ, BJ], [1, BK]]
        )
        a.offset = (2 + di) * SI + 2 * SJ + 2
        return a

    def tvi(t, B, di):
        a = t[:].copy()
        a.ap = bass_rust.VecI64Pair(
            [part_pair(t[:]), [FD, B], [BK, BJ], [1, BK]]
        )
        a.offset = di * BJ * BK
        return a

    groups = []
    for s0 in range(-2, 3):
        for s1 in range(-2, 3):
            if s0 == 0 and s1 == 0:
                groups.append((0, 0, 2, 2))
                groups.append((0, 0, -1, 2))
            else:
                groups.append((s0, s1, 2, 5))

    # Tiles read inside If_g but re-allocated by the pipelined always-part of
    # g+1 (emitted before If_g) need bufs=2.
    TAGS = {
        "dx": 2, "dy": 2, "dz": 2, "q": 2,
        "dvx": 1, "dvy": 1, "dvz": 1,
        "sx": 1, "sy": 1, "v": 1, "rr": 1, "cs": 1,
    }

    with tile.TileContext(nc) as tc:
        import contextlib
        with contextlib.ExitStack() as ctx:
            ipool = ctx.enter_context(tc.tile_pool(name="in", bufs=1))
            dpool = ctx.enter_context(tc.tile_pool(name="tmp", bufs=1))
            opool = ctx.enter_context(tc.tile_pool(name="out", bufs=1))
            ppool = ctx.enter_context(tc.tile_pool(name="psum", bufs=1, space="PSUM"))
            cpool = ctx.enter_context(tc.tile_pool(name="const", bufs=1))
            spool = ctx.enter_context(tc.tile_pool(name="small", bufs=2))

            ident_f = cpool.tile([128, 128], F32, name="ident_f")
            nc.sync.dma_start(ident_f[:], ident_d.ap())
            ident_s = cpool.tile([128, 128], F32R, name="ident_s")
            nc.vector.tensor_copy(ident_s[:], ident_f[:])
            zero_f = dpool.tile([128, FD], F32, tag="v", bufs=1,
                                name="zero_f", padded_shape=[128, 5 * FD])
            nc.vector.memset(zero_f[:], 0.0)
            zero_r = cpool.tile([128, FD], F32R, name="zero_r")
            nc.vector.tensor_copy(zero_r[:], zero_f[:])
            ones_f = cpool.tile([128, 1], F32, name="ones_f")
            nc.vector.memset(ones_f[:], 1.0)
            OrderedSet = bass.OrderedSet
            skip_regs = [nc.alloc_registers(
                f"skipregs{i}",
                OrderedSet([mybir.EngineType.DVE,
                            mybir.EngineType.Activation, mybir.EngineType.PE]),
            ) for i in range(2)]

            def body(r):
                tin = {}
                for nm in NAMES:
                    t = ipool.tile([128, BLKE], F32, tag=nm, name=f"in_{nm}")
                    src = dram_in[nm].ap()[bass.ds(r, 1), :, :]
                    nc.sync.dma_start(t[:].rearrange("p (u f) -> p u f", u=1), src)
                    tin[nm] = t

                accs = [ppool.tile([128, FD], F32, tag=f"acc{i}", name=f"acc{i}")
                        for i in range(6)]
                for ai in range(6):  # open accumulation groups (clear PSUM banks)
                    nc.tensor.matmul(accs[ai][:], ident_s[:], zero_r[:],
                                     start=True, stop=False, skip_group_check=True)

                def accum(ai, src_ap):
                    nc.tensor.matmul(accs[ai][:], ident_s[:], src_ap,
                                     start=False, stop=False, skip_group_check=True)

                V, G, A = nc.vector, nc.gpsimd, nc.scalar

                def mkt(tag, B, name, dtype=F32):
                    return dpool.tile([128, B * FD], dtype, tag=tag,
                                      bufs=TAGS[tag], name=name,
                                      padded_shape=[128, 5 * FD])

                spool_i = [0]

                def small(name):
                    spool_i[0] += 1
                    return spool.tile([128, 1], F32, tag=f"s{spool_i[0] % 4}",
                                      name=f"{name}{spool_i[0]}")

                def emit_always(gi, s0, s1, s2hi, B):
                    dx = mkt("dx", B, "dx")
                    dy = mkt("dy", B, "dy")
                    dz = mkt("dz", B, "dz")
                    for di in range(BI):
                        V.tensor_tensor(tvi(dx, B, di), basei(tin["x"], B, di), wini(tin["x"], s0, s1, 2 - s2hi, B, di), OP.subtract)
                    for di in range(BI):
                        V.tensor_tensor(tvi(dy, B, di), basei(tin["y"], B, di), wini(tin["y"], s0, s1, 2 - s2hi, B, di), OP.subtract)
                    for di in range(BI):
                        V.tensor_tensor(tvi(dz, B, di), basei(tin["z"], B, di), wini(tin["z"], s0, s1, 2 - s2hi, B, di), OP.subtract)
                    sx = mkt("sx", B, "sx")
                    sy = mkt("sy", B, "sy")
                    q = mkt("q", B, "q")
                    A.activation(sx[:], dx[:], AF.Square)
                    A.activation(sy[:], dy[:], AF.Square)
                    A.activation(q[:], dz[:], AF.Square)
                    G.tensor_tensor(sx[:], sx[:], sy[:], OP.add)
                    V.tensor_tensor(q[:], q[:], sx[:], OP.add)
                    junk = mkt("cs", B, "junk")
                    msum = small("msum")
                    V.tensor_scalar(junk[:], q[:], fourd2, 0.0, OP.is_lt, OP.add,
                                    accum_out=msum[:])
                    pp = ppool.tile([1, 1], F32, tag="hit", name="hit", bufs=2)
                    nc.tensor.matmul(pp[:1, :1], ones_f[:, :1], msum[:, :1],
                                     start=True, stop=True, skip_group_check=True)
                    hits = small("hits")
                    V.tensor_copy(hits[:1, :1], pp[:1, :1])
                    regs = skip_regs[gi % 2]
                    nc.regs_load(regs, hits[:1, :1].bitcast(mybir.dt.int32))
                    return regs, dx, dy, dz, q

                def emit_hot(state, s0, s1, s2hi, B):
                    regs, dx, dy, dz, q = state
                    with tc.If(nc.snap(regs) > 0):
                        l = mkt("sx", B, "l")
                        A.activation(l[:], q[:], AF.Ln)
                        r0 = mkt("sy", B, "r0")
                        A.activation(r0[:], l[:], AF.Exp, scale=-0.5)
                        u = mkt("cs", B, "u")
                        A.activation(u[:], r0[:], AF.Square)
                        v = mkt("v", B, "v")
                        V.tensor_tensor(v[:], q[:], u[:], OP.mult)
                        V.tensor_scalar(v[:], v[:], -0.5, 1.5, OP.mult, OP.add)
                        rr = mkt("rr", B, "rr")
                        V.tensor_tensor(rr[:], r0[:], v[:], OP.mult)
                        cv = mkt("sy", B, "cv")
                        V.tensor_scalar(cv[:], rr[:], -float(two_d), 1.0, OP.mult, OP.add)
                        cs = mkt("cs", B, "cs")
                        V.scalar_tensor_tensor(cs[:], q[:], fourd2, cv[:], OP.is_lt, OP.mult)
                        dvx = mkt("dvx", B, "dvx")
                        dvy = mkt("dvy", B, "dvy")
                        dvz = mkt("dvz", B, "dvz")
                        for di in range(BI):
                            V.tensor_tensor(tvi(dvx, B, di), basei(tin["vx"], B, di), wini(tin["vx"], s0, s1, 2 - s2hi, B, di), OP.subtract)
                        for di in range(BI):
                            V.tensor_tensor(tvi(dvy, B, di), basei(tin["vy"], B, di), wini(tin["vy"], s0, s1, 2 - s2hi, B, di), OP.subtract)
                        for di in range(BI):
                            V.tensor_tensor(tvi(dvz, B, di), basei(tin["vz"], B, di), wini(tin["vz"], s0, s1, 2 - s2hi, B, di), OP.subtract)
                        p1 = mkt("v", B, "p1")
                        p2 = mkt("sy", B, "p2")
                        p3 = mkt("sx", B, "p3")
                        V.tensor_tensor(p1[:], dvx[:], dx[:], OP.mult)
                        V.tensor_tensor(p2[:], dvy[:], dy[:], OP.mult)
                        V.tensor_tensor(p3[:], dvz[:], dz[:], OP.mult)
                        V.tensor_tensor(p1[:], p1[:], p2[:], OP.add)
                        V.tensor_tensor(p1[:], p1[:], p3[:], OP.add)
                        r2 = mkt("sx", B, "r2")
                        A.activation(r2[:], rr[:], AF.Square)
                        wd = mkt("sy", B, "wd")
                        V.tensor_tensor(wd[:], p1[:], r2[:], OP.mult)
                        bs = mkt("dvx", B, "bs")
                        V.scalar_tensor_tensor(bs[:], q[:], fourd2, wd[:], OP.is_lt, OP.mult)
                        # damping fields first (frees bs slot), then collision
                        ex = mkt("rr", B, "ex", F32R)
                        ey = mkt("sy", B, "ey", F32R)
                        ez = mkt("sx", B, "ez", F32R)
                        V.tensor_tensor(ex[:], bs[:], dx[:], OP.mult)
                        V.tensor_tensor(ey[:], bs[:], dy[:], OP.mult)
                        V.tensor_tensor(ez[:], bs[:], dz[:], OP.mult)
                        cx = mkt("dvx", B, "cx", F32R)
                        cy = mkt("dvy", B, "cy", F32R)
                        cz = mkt("dvz", B, "cz", F32R)
                        V.tensor_tensor(cx[:], cs[:], dx[:], OP.mult)
                        V.tensor_tensor(cy[:], cs[:], dy[:], OP.mult)
                        V.tensor_tensor(cz[:], cs[:], dz[:], OP.mult)
                        for m in range(B):
                            sl = slice(m * FD, (m + 1) * FD)
                            accum(3, ex[:, sl])
                            accum(4, ey[:, sl])
                            accum(5, ez[:, sl])
                            accum(0, cx[:, sl])
                            accum(1, cy[:, sl])
                            accum(2, cz[:, sl])

                pending = None
                for gi, (s0, s1, s2hi, B) in enumerate(groups):
                    state = emit_always(gi, s0, s1, s2hi, B)
                    if pending is not None:
                        emit_hot(*pending)
                    pending = (state, s0, s1, s2hi, B)
                emit_hot(*pending)

                for ai in range(6):  # close accumulation groups
                    nc.tensor.matmul(accs[ai][:], ident_s[:], zero_r[:],
                                     start=False, stop=True, skip_group_check=True)

                # ---- scale accumulators to output force values ----
                outs = []
                for o in range(6):
                    ot = opool.tile([128, FD], F32, tag=f"o{o}", name=f"o{o}")
                    A.activation(ot[:], accs[o][:], AF.Copy, scale=(KN if o < 3 else ETA))
                    outs.append(ot)

                # ---- friction from last shift (2,2,2), gated on last-shift hits ----
                dxl = mkt("dx", 1, "dxl")
                dyl = mkt("dy", 1, "dyl")
                dzl = mkt("dz", 1, "dzl")
                for di in range(BI):
                    V.tensor_tensor(tvi(dxl, 1, di), basei(tin["x"], 1, di), wini(tin["x"], 2, 2, 0, 1, di), OP.subtract)
                for di in range(BI):
                    V.tensor_tensor(tvi(dyl, 1, di), basei(tin["y"], 1, di), wini(tin["y"], 2, 2, 0, 1, di), OP.subtract)
                for di in range(BI):
                    V.tensor_tensor(tvi(dzl, 1, di), basei(tin["z"], 1, di), wini(tin["z"], 2, 2, 0, 1, di), OP.subtract)
                t1 = mkt("sx", 1, "t1")
                t2 = mkt("sy", 1, "t2")
                ql = mkt("q", 1, "ql")
                A.activation(t1[:], dxl[:], AF.Square)
                A.activation(t2[:], dyl[:], AF.Square)
                A.activation(ql[:], dzl[:], AF.Square)
                G.tensor_tensor(t1[:], t1[:], t2[:], OP.add)
                G.tensor_tensor(ql[:], ql[:], t1[:], OP.add)
                junkl = mkt("cs", 1, "junkl")
                msl = small("msl")
                V.tensor_scalar(junkl[:], ql[:], fourd2, 0.0, OP.is_lt, OP.add,
                                accum_out=msl[:])
                ppl = ppool.tile([1, 1], F32, tag="hit", name="hitl", bufs=2)
                nc.tensor.matmul(ppl[:1, :1], ones_f[:, :1], msl[:, :1],
                                 start=True, stop=True, skip_group_check=True)
                hitsl = small("hitsl")
                V.tensor_copy(hitsl[:1, :1], ppl[:1, :1])
                nc.regs_load(skip_regs[0], hitsl[:1, :1].bitcast(mybir.dt.int32))

                fr_outs = []
                for o in range(3):
                    ot = opool.tile([128, FD], F32, tag=f"o{6+o}", name=f"o{6+o}")
                    V.memset(ot[:], 0.0)
                    fr_outs.append(ot)

                with tc.If(nc.snap(skip_regs[0]) > 0):
                    mneg = mkt("cs", 1, "mneg")  # -(q < (2d)^2)
                    V.tensor_scalar(mneg[:], ql[:], fourd2, -1.0, OP.is_lt, OP.mult)
                    dvxl = mkt("dvx", 1, "dvxl")
                    dvyl = mkt("dvy", 1, "dvyl")
                    dvzl = mkt("dvz", 1, "dvzl")
                    for di in range(BI):
                        V.tensor_tensor(tvi(dvxl, 1, di), basei(tin["vx"], 1, di), wini(tin["vx"], 2, 2, 0, 1, di), OP.subtract)
                    for di in range(BI):
                        V.tensor_tensor(tvi(dvyl, 1, di), basei(tin["vy"], 1, di), wini(tin["vy"], 2, 2, 0, 1, di), OP.subtract)
                    for di in range(BI):
                        V.tensor_tensor(tvi(dvzl, 1, di), basei(tin["vz"], 1, di), wini(tin["vz"], 2, 2, 0, 1, di), OP.subtract)

                    def safe_recip_abs(dv, rtag, idx):
                        # 1 / max(EPS, |dv|), exp/ln + 2 Newton steps
                        aa = mkt("v", 1, f"aa{idx}")
                        A.activation(aa[:], dv[:], AF.Abs)
                        V.tensor_scalar(aa[:], aa[:], EPS, None, OP.max)
                        ll = mkt("q", 1, f"ll{idx}")
                        A.activation(ll[:], aa[:], AF.Ln)
                        rr0 = mkt(rtag, 1, f"rcp{idx}")
                        A.activation(rr0[:], ll[:], AF.Exp, scale=-1.0)
                        for it in range(2):
                            tn = mkt("q", 1, f"tn{idx}_{it}")
                            V.tensor_tensor(tn[:], aa[:], rr0[:], OP.mult)
                            V.tensor_scalar(tn[:], tn[:], -1.0, 2.0, OP.mult, OP.add)
                            V.tensor_tensor(rr0[:], rr0[:], tn[:], OP.mult)
                        return rr0

                    rx = safe_recip_abs(dvxl, "sx", 0)
                    ry = safe_recip_abs(dvyl, "sy", 1)
                    rz = safe_recip_abs(dvzl, "rr", 2)
                    # numerators: dvx/|dvx|_safe, dvy/|dvy|_safe, dvy/|dvz|_safe
                    fax = mkt("dx", 1, "fax")
                    fay = mkt("dy", 1, "fay")
                    faz = mkt("dz", 1, "faz")
                    V.tensor_tensor(fax[:], dvxl[:], rx[:], OP.mult)
                    V.tensor_tensor(fay[:], dvyl[:], ry[:], OP.mult)
                    V.tensor_tensor(faz[:], dvyl[:], rz[:], OP.mult)

                    afx = mkt("dvx", 1, "afx")
                    afy = mkt("dvy", 1, "afy")
                    afz = mkt("dvz", 1, "afz")
                    A.activation(afx[:], outs[0][:], AF.Abs)
                    A.activation(afy[:], outs[1][:], AF.Abs)
                    A.activation(afz[:], outs[2][:], AF.Abs)

                    fr_defs = [
                        (afy, afz, outs[3], fax),
                        (afx, afz, outs[4], fay),
                        (afx, afy, outs[5], faz),
                    ]
                    for o, (a1, a2, dmp, fac) in enumerate(fr_defs):
                        u1 = mkt("v", 1, f"u1_{o}")
                        V.tensor_tensor(u1[:], a1[:], a2[:], OP.add)
                        V.tensor_tensor(u1[:], u1[:], dmp[:], OP.subtract)
                        u2 = mkt("q", 1, f"u2_{o}")
                        A.activation(u2[:], u1[:], AF.Abs, scale=MU)
                        V.tensor_tensor(u2[:], u2[:], fac[:], OP.mult)
                        V.tensor_tensor(fr_outs[o][:], u2[:], mneg[:], OP.mult)
                outs.extend(fr_outs)

                for o in range(9):
                    dst = out_d.ap()[bass.ds(r, 1), o, :, :]
                    nc.sync.dma_start(dst, outs[o][:].rearrange("p (u f) -> p u f", u=1))

            with tc.For_i(0, ROUNDS, 1) as r:
                body(r)

    return nc


def _split_excess_waits(nc, max_waits=1):
    """This walrus build allows only max_waits semaphore waits per instruction;
    hoist the excess onto NoOps inserted just before the offender."""
    from concourse import mybir
    cnt = 0
    for blk in nc.m.functions[0].blocks:
        new = []
        changed = False
        for ins in blk.instructions:
            si = ins.sync_info
            if si is not None and si.on_wait and len(si.on_wait) > max_waits:
                waits = list(si.on_wait)
                keep = waits[-max_waits:]
                extra = waits[:-max_waits]
                for i in range(0, len(extra), max_waits):
                    nop = mybir.InstNoOp(name=f"wait_split_{cnt}", ins=[], outs=[])
                    cnt += 1
                    nop.engine = ins.engine
                    nop.sync_info = type(si)(on_wait=extra[i:i + max_waits], on_update=[])
                    new.append(nop)
                ins.sync_info = type(si)(on_wait=keep, on_update=si.on_update)
                changed = True
            new.append(ins)
        if changed:
            blk.instructions = new
    return nc


# ---------------- host-side sharding ----------------

def _prep_in_maps(inputs):
    from numpy.lib.stride_tricks import sliding_window_view

    arrs = {
        "x": inputs["x_grid"], "y": inputs["y_grid"], "z": inputs["z_grid"],
        "vx": inputs["vx_grid"], "vy": inputs["vy_grid"], "vz": inputs["vz_grid"],
    }
    lanes = np.arange(128)
    j_starts = (lanes % JBLK) * BJ
    k_starts = np.arange(ROUNDS) * BK
    ident = np.eye(128, dtype=np.float32)

    in_maps = [dict() for _ in range(CORES)]
    for nm, a in arrs.items():
        ap = np.pad(np.asarray(a, dtype=np.float32), 2, mode="wrap")  # [196]^3
        W = sliding_window_view(ap, (HI, HJ, HK))
        for c in range(CORES):
            i_starts = c * NIP + (lanes // JBLK) * BI
            blk = W[i_starts[None, :], j_starts[None, :], k_starts[:, None]]
            in_maps[c][nm] = np.ascontiguousarray(
                blk.reshape(ROUNDS, 128, BLKE), dtype=np.float32)
    for c in range(CORES):
        in_maps[c]["ident"] = ident
    return in_maps


def _gather(results):
    out = np.empty((9, N, N, N), dtype=np.float32)
    for c in range(CORES):
        r = results[c]["out"].reshape(ROUNDS, 9, IBLK, JBLK, BI, BJ, BK)
        # [r, o, ib, jb, di, dj, dk] -> [o, ib, di, jb, dj, r, dk]
        blk = r.transpose(1, 2, 4, 3, 5, 0, 6).reshape(9, NIP, N, N)
        out[:, c * NIP:(c + 1) * NIP] = blk
    return out


def _run(inputs, trace=False):
    from concourse.bass_utils import run_bass_kernel_spmd

    d = float(np.asarray(inputs["d"]))
    two_d = 2.0 * d
    key = round(two_d, 9)
    if key not in _cache:
        _cache[key] = _split_excess_waits(_build_nc(two_d))
    nc = _cache[key]
    in_maps = _prep_in_maps(inputs)
    res = run_bass_kernel_spmd(nc, in_maps, core_ids=list(range(CORES)), trace=trace)
    return _gather(res.results), res


def kernel(**inputs):
    out, _ = _run(inputs, trace=False)
    return out
